# revision 1
# baseline (speedup 1.0000x reference)
"""Graph-Transformer message-passing kernel for 8 Trainium2 NeuronCores.

Strategy (1D dst-shard edge parallelism):
  - Nodes are split into 8 contiguous ranges; core c owns all edges whose dst
    falls in its range, so segment max/sum/aggregation are fully local.
  - Host groups each core's dst nodes into degree classes (W in {4,8,16,32,64}),
    pads each node's in-edge list to W slots (pad slots get bias -1e30 so they
    vanish in softmax), and lays nodes out in a per-core "perm" order so the
    on-device pipeline is fully regular.
  - Per layer, each core projects Q|K|V for its own nodes (PE), the q|v halves
    are AllGathered into a global qv table in DRAM, and each node-tile then
    indirect-DMA-gathers its W source rows, runs the masked edge softmax and
    weighted aggregation on DVE/ACT, and applies the output projection (PE).
  - 3 layers are fused in one NEFF; output rows return in perm order and the
    host inverts the permutation.
"""

import numpy as np

import concourse.bass as bass
import concourse.bacc as bacc
import concourse.mybir as mybir
import concourse.tile as tile
from concourse.bass import IndirectOffsetOnAxis
from concourse.masks import make_identity
from concourse.bass_utils import run_bass_kernel_spmd

NCORES = 8
L = 3
H = 8
D = 128
HD = D // H
SCALE = 1.0 / float(np.sqrt(HD))
NEG = -1.0e30
P = 128

FP = mybir.dt.float32
I32 = mybir.dt.int32
AX = mybir.AxisListType
OP = mybir.AluOpType


# ----------------------------------------------------------------------------
# Host-side layout
# ----------------------------------------------------------------------------

class Layout:
    pass


def build_layout(src, dst, n_nodes):
    """Group each core's dst nodes by degree class and build gather tables."""
    src = np.asarray(src).astype(np.int64)
    dst = np.asarray(dst).astype(np.int64)
    N = n_nodes
    chunk = (N + NCORES - 1) // NCORES

    deg = np.bincount(dst, minlength=N)
    order = np.argsort(dst, kind="stable")
    src_sorted = src[order]
    starts = np.zeros(N + 1, dtype=np.int64)
    np.cumsum(deg, out=starts[1:])

    max_deg = int(deg.max())
    w_all = [4, 8, 16, 32, 64]
    assert max_deg <= w_all[-1], f"max degree {max_deg} exceeds supported 64"
    classes = []
    lo = 0
    for w in w_all:
        sel = (deg > lo) & (deg <= w)
        if sel.any():
            classes.append(w)
        lo = w

    # per-core per-class node lists
    node_lists = {}  # (core, W) -> ascending node ids
    for c in range(NCORES):
        nlo, nhi = c * chunk, min(N, (c + 1) * chunk)
        d = deg[nlo:nhi]
        lo = 0
        for w in classes:
            sel = np.nonzero((d > lo) & (d <= w))[0] + nlo
            node_lists[(c, w)] = sel
            lo = w

    # identical per-class capacity on every core, in whole 128-node tiles
    caps = {}
    for w in classes:
        cap = max(len(node_lists[(c, w)]) for c in range(NCORES))
        caps[w] = ((cap + P - 1) // P) * P

    n_class_rows = sum(caps.values())
    deg0_max = max(
        ((min(N, (c + 1) * chunk) - c * chunk) - sum(len(node_lists[(c, w)]) for w in classes))
        for c in range(NCORES)
    )
    R = ((n_class_rows + max(deg0_max, 0) + P - 1) // P) * P
    T = R // P

    # perm order per core + global row index of every node
    perm = np.full((NCORES, R), -1, dtype=np.int64)
    row_of = np.full(N, -1, dtype=np.int64)
    base = {}
    b = 0
    for w in classes:
        base[w] = b
        b += caps[w]
    for c in range(NCORES):
        nlo, nhi = c * chunk, min(N, (c + 1) * chunk)
        for w in classes:
            nl = node_lists[(c, w)]
            perm[c, base[w]:base[w] + len(nl)] = nl
            row_of[nl] = c * R + base[w] + np.arange(len(nl))
        deg0 = np.nonzero(deg[nlo:nhi] == 0)[0] + nlo
        perm[c, n_class_rows:n_class_rows + len(deg0)] = deg0
        # deg0 rows need correct x (they may be gathered as src)
        row_of[deg0] = c * R + n_class_rows + np.arange(len(deg0))

    # gather index + bias tables, per class, per core
    idx_tabs = {}   # (c, w) -> [capW, w] int32 rows into global qv table
    bias_tabs = {}
    for c in range(NCORES):
        for w in classes:
            cap = caps[w]
            it = np.zeros((cap, w), dtype=np.int32)
            bt = np.full((cap, w), NEG, dtype=np.float32)
            nl = node_lists[(c, w)]
            for i, g in enumerate(nl):
                d = int(deg[g])
                srcs = src_sorted[starts[g]:starts[g] + d]
                it[i, :d] = row_of[srcs].astype(np.int32)
                bt[i, :d] = 0.0
            idx_tabs[(c, w)] = it
            bias_tabs[(c, w)] = bt

    lay = Layout()
    lay.N, lay.R, lay.T, lay.chunk = N, R, T, chunk
    lay.classes, lay.caps, lay.base = classes, caps, base
    lay.n_class_tiles = n_class_rows // P
    lay.perm, lay.row_of = perm, row_of
    lay.idx_tabs, lay.bias_tabs = idx_tabs, bias_tabs
    return lay


def host_inputs(lay, x, qkv_w, qkv_b, out_w, out_b, out_w_last, out_b_last):
    """Build the per-core in_maps."""
    x = np.asarray(x, dtype=np.float32)
    nclass = out_w_last.shape[1]
    wc = np.zeros((L, D, 3 * D), dtype=np.float32)
    bc = np.zeros((L, P, 3 * D), dtype=np.float32)
    wo = np.zeros((L, D, D), dtype=np.float32)
    bo = np.zeros((L, P, D), dtype=np.float32)
    for l in range(L):
        wq, wk, wv = qkv_w[l, 0], qkv_w[l, 1], qkv_w[l, 2]
        bq, bk, bv = qkv_b[l, 0], qkv_b[l, 1], qkv_b[l, 2]
        wc[l] = np.concatenate([wq, wk * SCALE, wv], axis=1)
        bcl = np.concatenate([bq, bk * SCALE, bv])
        bc[l] = np.tile(bcl[None, :], (P, 1))
        if l < L - 1:
            wo[l] = out_w[l]
            bo[l] = np.tile(out_b[l][None, :], (P, 1))
        else:
            wo[l, :, :nclass] = out_w_last
            bo[l, :, :nclass] = np.tile(out_b_last[None, :], (P, 1))

    in_maps = []
    for c in range(NCORES):
        m = {
            "x0": np.where(
                (lay.perm[c] >= 0)[:, None], x[np.maximum(lay.perm[c], 0)], 0.0
            ).astype(np.float32),
            "wc": wc, "bc": bc, "wo": wo, "bo": bo,
        }
        for w in lay.classes:
            m[f"idx{w}"] = lay.idx_tabs[(c, w)]
            m[f"bias{w}"] = lay.bias_tabs[(c, w)]
        in_maps.append(m)
    return in_maps


def host_output(lay, outs, nclass):
    """Invert the perm: outs is list of [R, nclass] per core."""
    full = np.zeros((lay.N, nclass), dtype=np.float32)
    for c in range(NCORES):
        real = lay.perm[c] >= 0
        full[lay.perm[c][real]] = outs[c][real]
    return full


# ----------------------------------------------------------------------------
# Device program
# ----------------------------------------------------------------------------

def build_nc(lay, nclass):
    R, T = lay.R, lay.T
    nc = bacc.Bacc(trn_type="TRN2", num_devices=NCORES)

    x0 = nc.dram_tensor("x0", [R, D], FP, kind="ExternalInput")
    wc = nc.dram_tensor("wc", [L, D, 3 * D], FP, kind="ExternalInput")
    bc = nc.dram_tensor("bc", [L, P, 3 * D], FP, kind="ExternalInput")
    wo = nc.dram_tensor("wo", [L, D, D], FP, kind="ExternalInput")
    bo = nc.dram_tensor("bo", [L, P, D], FP, kind="ExternalInput")
    idx_d, bias_d = {}, {}
    for w in lay.classes:
        cap = lay.caps[w]
        idx_d[w] = nc.dram_tensor(f"idx{w}", [cap, w], I32, kind="ExternalInput")
        bias_d[w] = nc.dram_tensor(f"bias{w}", [cap, w], FP, kind="ExternalInput")
    out_ext = nc.dram_tensor("out", [R, nclass], FP, kind="ExternalOutput")

    qv_slice = nc.dram_tensor("qv_slice", [R, 2 * D], FP, kind="Internal")
    qv_full = nc.dram_tensor(
        "qv_full", [NCORES * R, 2 * D], FP, kind="Internal", addr_space="Shared"
    )
    rg = [list(range(NCORES))]

    with tile.TileContext(nc) as tc:
        with (
            tc.tile_pool(name="const", bufs=1) as cpool,
            tc.tile_pool(name="persist", bufs=1) as ppool,
            tc.tile_pool(name="proj", bufs=3) as projpool,
            tc.tile_pool(name="work", bufs=2) as wpool,
            tc.tile_pool(name="small", bufs=3) as spool,
            tc.tile_pool(name="psum", bufs=2, space="PSUM") as pspool,
            tc.tile_pool(name="psum_o", bufs=2, space="PSUM") as pspool_o,
        ):
            ident = cpool.tile([P, P], FP, tag="ident", name="ident")
            make_identity(nc, ident[:])
            wc_sb = cpool.tile([P, L * 3 * D], FP, tag="wc", name="wc")
            nc.sync.dma_start(wc_sb[:].rearrange("k (l n) -> k l n", l=L), wc[:].rearrange("l k n -> k l n"))
            bc_sb = cpool.tile([P, L * 3 * D], FP, tag="bc", name="bc")
            nc.sync.dma_start(bc_sb[:].rearrange("p (l n) -> p l n", l=L), bc[:].rearrange("l p n -> p l n"))
            wo_sb = cpool.tile([P, L * D], FP, tag="wo", name="wo")
            nc.sync.dma_start(wo_sb[:].rearrange("k (l n) -> k l n", l=L), wo[:].rearrange("l k n -> k l n"))
            bo_sb = cpool.tile([P, L * D], FP, tag="bo", name="bo")
            nc.sync.dma_start(bo_sb[:].rearrange("p (l n) -> p l n", l=L), bo[:].rearrange("l p n -> p l n"))

            idx_sb, bias_sb = {}, {}
            for w in lay.classes:
                tw = lay.caps[w] // P
                idx_sb[w] = cpool.tile([P, tw * w], I32, tag=f"idx{w}", name=f"idx{w}")
                nc.sync.dma_start(
                    idx_sb[w][:].rearrange("p (t w) -> p t w", w=w),
                    idx_d[w][:].rearrange("(t p) w -> p t w", p=P),
                )
                bias_sb[w] = cpool.tile([P, tw * w], FP, tag=f"bias{w}", name=f"bias{w}")
                nc.sync.dma_start(
                    bias_sb[w][:].rearrange("p (t w) -> p t w", w=w),
                    bias_d[w][:].rearrange("(t p) w -> p t w", p=P),
                )

            x_sb = ppool.tile([P, T * D], FP, tag="x", name="x")
            nc.sync.dma_start(x_sb[:].rearrange("p (t f) -> p t f", f=D), x0[:].rearrange("(t p) f -> p t f", p=P))
            k_sb = ppool.tile([P, T * D], FP, tag="k", name="k")
            tc.strict_bb_all_engine_barrier()

            for l in range(L):
                # ---- QKV projection for own rows ----
                for t in range(T):
                    xT_ps = pspool.tile([P, P], FP, tag="xT", name="xT")
                    nc.tensor.transpose(
                        xT_ps[:], x_sb[:, t * D:(t + 1) * D], ident[:]
                    )
                    xT = projpool.tile([P, P], FP, tag="xT_sb", name="xT_sb")
                    nc.vector.tensor_copy(xT[:], xT_ps[:])
                    qkv_ps = pspool.tile([P, 3 * D], FP, tag="qkv", name="qkv")
                    nc.tensor.matmul(
                        qkv_ps[:], lhsT=xT[:],
                        rhs=wc_sb[:, l * 3 * D:(l + 1) * 3 * D],
                        start=True, stop=True,
                    )
                    qv_st = projpool.tile([P, 2 * D], FP, tag="qv_st", name="qv_st")
                    bofs = l * 3 * D
                    nc.vector.tensor_tensor(
                        out=qv_st[:, 0:D], in0=qkv_ps[:, 0:D],
                        in1=bc_sb[:, bofs:bofs + D], op=OP.add,
                    )
                    nc.vector.tensor_tensor(
                        out=qv_st[:, D:2 * D], in0=qkv_ps[:, 2 * D:3 * D],
                        in1=bc_sb[:, bofs + 2 * D:bofs + 3 * D], op=OP.add,
                    )
                    nc.vector.tensor_tensor(
                        out=k_sb[:, t * D:(t + 1) * D], in0=qkv_ps[:, D:2 * D],
                        in1=bc_sb[:, bofs + D:bofs + 2 * D], op=OP.add,
                    )
                    nc.sync.dma_start(qv_slice[t * P:(t + 1) * P, :], qv_st[:])

                # ---- exchange qv ----
                nc.gpsimd.collective_compute(
                    "AllGather", OP.bypass, replica_groups=rg,
                    ins=[qv_slice[:, :]], outs=[qv_full[:, :]],
                )
                tc.strict_bb_all_engine_barrier()

                # ---- per-class edge softmax + aggregation + out projection ----
                nt = 0
                for w in lay.classes:
                    tw = lay.caps[w] // P
                    for t in range(tw):
                        qv_g = wpool.tile([P, w * 2 * D], FP, tag="qvg", name=f"qvg{w}")
                        # HW indirect DMA consumes ONE offset per partition and
                        # reads the dest's free size contiguously from that row
                        # (interp's per-chunk-offset semantics do not hold), so
                        # issue one gather per edge slot.
                        for ws in range(w):
                            nc.gpsimd.indirect_dma_start(
                                out=qv_g[:, ws * 2 * D:(ws + 1) * 2 * D],
                                out_offset=None,
                                in_=qv_full[:, :],
                                in_offset=IndirectOffsetOnAxis(
                                    ap=idx_sb[w][:, t * w + ws:t * w + ws + 1], axis=0
                                ),
                            )
                        q_ap = qv_g[:].rearrange("p (w c) -> p w c", c=2 * D)[:, :, 0:D]
                        v_ap = qv_g[:].rearrange("p (w c) -> p w c", c=2 * D)[:, :, D:2 * D]
                        k_ap = (
                            k_sb[:, nt * D:(nt + 1) * D]
                            .unsqueeze(1).to_broadcast([P, w, D])
                        )
                        tmp = wpool.tile([P, w * D], FP, tag="tmp", name=f"tmp{w}")
                        nc.vector.tensor_tensor(
                            out=tmp[:].rearrange("p (w f) -> p w f", f=D),
                            in0=q_ap, in1=k_ap, op=OP.mult,
                        )
                        s = spool.tile([P, w * H], FP, tag="s", name=f"s{w}")
                        nc.vector.reduce_sum(
                            s[:].rearrange("p (w h) -> p w h", h=H),
                            tmp[:].rearrange("p (w h d) -> p w h d", h=H, d=HD),
                            axis=AX.X,
                        )
                        bia = (
                            bias_sb[w][:, t * w:(t + 1) * w]
                            .unsqueeze(2).to_broadcast([P, w, H])
                        )
                        nc.vector.tensor_tensor(
                            out=s[:].rearrange("p (w h) -> p w h", h=H),
                            in0=s[:].rearrange("p (w h) -> p w h", h=H),
                            in1=bia, op=OP.add,
                        )
                        smax = spool.tile([P, H], FP, tag="smax", name="smax")
                        nc.vector.reduce_max(
                            smax[:],
                            s[:].rearrange("p (w h) -> p h w", h=H),
                            axis=AX.X,
                        )
                        ex = spool.tile([P, w * H], FP, tag="ex", name=f"ex{w}")
                        nc.vector.tensor_tensor(
                            out=ex[:].rearrange("p (w h) -> p w h", h=H),
                            in0=s[:].rearrange("p (w h) -> p w h", h=H),
                            in1=smax[:].unsqueeze(1).to_broadcast([P, w, H]),
                            op=OP.subtract,
                        )
                        nc.scalar.activation(
                            out=ex[:], in_=ex[:],
                            func=mybir.ActivationFunctionType.Exp,
                        )
                        denom = spool.tile([P, H], FP, tag="denom", name="denom")
                        nc.vector.reduce_sum(
                            denom[:],
                            ex[:].rearrange("p (w h) -> p h w", h=H),
                            axis=AX.X,
                        )
                        rec = spool.tile([P, H], FP, tag="rec", name="rec")
                        nc.vector.reciprocal(rec[:], denom[:])
                        probs = spool.tile([P, w * H], FP, tag="probs", name=f"probs{w}")
                        nc.vector.tensor_tensor(
                            out=probs[:].rearrange("p (w h) -> p w h", h=H),
                            in0=ex[:].rearrange("p (w h) -> p w h", h=H),
                            in1=rec[:].unsqueeze(1).to_broadcast([P, w, H]),
                            op=OP.mult,
                        )
                        nc.vector.tensor_tensor(
                            out=tmp[:].rearrange("p (w h d) -> p w h d", h=H, d=HD),
                            in0=v_ap.rearrange("p w (h d) -> p w h d", h=H),
                            in1=probs[:].rearrange("p (w h) -> p w h", h=H).unsqueeze(3)
                                .to_broadcast([P, w, H, HD]),
                            op=OP.mult,
                        )
                        agg = projpool.tile([P, D], FP, tag="agg", name="agg")
                        nc.vector.reduce_sum(
                            agg[:],
                            tmp[:].rearrange("p (w f) -> p f w", f=D),
                            axis=AX.X,
                        )
                        # fused output projection for this node tile
                        aT_ps = pspool.tile([P, P], FP, tag="aT", name="aT")
                        nc.tensor.transpose(aT_ps[:], agg[:], ident[:])
                        aT = projpool.tile([P, P], FP, tag="aT_sb", name="aT_sb")
                        nc.vector.tensor_copy(aT[:], aT_ps[:])
                        o_ps = pspool_o.tile([P, D], FP, tag="o", name="o")
                        nc.tensor.matmul(
                            o_ps[:], lhsT=aT[:], rhs=wo_sb[:, l * D:(l + 1) * D],
                            start=True, stop=True,
                        )
                        if l < L - 1:
                            nc.vector.tensor_tensor(
                                out=x_sb[:, nt * D:(nt + 1) * D], in0=o_ps[:],
                                in1=bo_sb[:, l * D:(l + 1) * D], op=OP.add,
                            )
                        else:
                            o_sb = projpool.tile([P, nclass], FP, tag="o_sb", name="o_sb")
                            nc.vector.tensor_tensor(
                                out=o_sb[:], in0=o_ps[:, 0:nclass],
                                in1=bo_sb[:, l * D:l * D + nclass], op=OP.add,
                            )
                            nc.sync.dma_start(
                                out_ext[nt * P:(nt + 1) * P, :], o_sb[:]
                            )
                        nt += 1
                # tail tiles (deg-0 + padding rows): out = bias only
                for t in range(lay.n_class_tiles, T):
                    if l < L - 1:
                        nc.vector.tensor_copy(
                            x_sb[:, t * D:(t + 1) * D], bo_sb[:, l * D:(l + 1) * D]
                        )
                    else:
                        o_sb = projpool.tile([P, nclass], FP, tag="o_sb", name="o_sb")
                        nc.vector.tensor_copy(
                            o_sb[:], bo_sb[:, l * D:l * D + nclass]
                        )
                        nc.sync.dma_start(out_ext[t * P:(t + 1) * P, :], o_sb[:])
    nc.compile()
    return nc


# ----------------------------------------------------------------------------
# Entry point
# ----------------------------------------------------------------------------

_trace = [False]  # test.py can flip this to profile


def kernel(x, src, dst, qkv_w, qkv_b, out_w, out_b, out_w_last, out_b_last):
    x = np.asarray(x, dtype=np.float32)
    lay = build_layout(np.asarray(src), np.asarray(dst), x.shape[0])
    nclass = np.asarray(out_w_last).shape[1]
    in_maps = host_inputs(
        lay, x, np.asarray(qkv_w, dtype=np.float32),
        np.asarray(qkv_b, dtype=np.float32), np.asarray(out_w, dtype=np.float32),
        np.asarray(out_b, dtype=np.float32), np.asarray(out_w_last, dtype=np.float32),
        np.asarray(out_b_last, dtype=np.float32),
    )
    nc = build_nc(lay, nclass)
    res = run_bass_kernel_spmd(
        nc, in_maps, core_ids=list(range(NCORES)), trace=_trace[0]
    )
    kernel.last_results = res
    outs = [res.results[c]["out"] for c in range(NCORES)]
    return host_output(lay, outs, nclass)



# revision 9
# speedup vs baseline: 1.1117x; 1.1117x over previous
"""Graph-Transformer message-passing kernel for 8 Trainium2 NeuronCores.

Strategy (v2, dst-shard + batched SWDGE gathers, bf16):
  - Nodes split into 8 contiguous dst ranges; core c owns all edges into its
    range, so softmax/aggregation are fully local to a core.
  - Host groups each core's nodes into degree classes (DP-optimized widths),
    pads each node's in-edge list to the class width W; pad slots carry a
    -1e30 mask so they vanish in the softmax.
  - Per layer: each core projects Q|K|V for its rows (PE, bf16), the q|v
    halves are AllGathered into a global [8R, 256] bf16 table, each core then
    compacts the ~31.7K unique source rows it needs into a private <32768-row
    table (two dma_gather passes, int16 idx limit), and edge-slot q|v rows are
    batch-gathered from it (one dma_gather per ~32-slot chunk instead of one
    indirect DMA per edge slot: ~1us fixed SWDGE cost amortized 32x).
  - Masked softmax (no max-subtraction: |score| < 4 for this model family) and
    weighted aggregation run on DVE in bf16; output projection on PE keeps x
    feature-major (xT) so the next layer's QKV needs no transpose.
  - 3 layers fused in one NEFF; host inverts the node permutation.
"""

import numpy as np
import ml_dtypes

import concourse.bass as bass
import concourse.bacc as bacc
import concourse.mybir as mybir
import concourse.tile as tile
from concourse.masks import make_identity
from concourse.bass_utils import run_bass_kernel_spmd

NCORES = 8
L = 3
H = 8
D = 128
HD = D // H
SCALE = 1.0 / float(np.sqrt(HD))
NEG = -1.0e30
P = 128
LO_ROWS = 32768          # int16 idx limit for the low gather region
SLOTS_PER_CHUNK = 32     # edge-slot columns per dma_gather / DVE chunk
COMP_CHUNK = 1024        # rows per compaction dma_gather (SWDGE ring: <=65 descs/ring)
GATHER_MAX_SLOTS = 8     # 128*8 = 1024 idxs per edge dma_gather

FP = mybir.dt.float32
BF = mybir.dt.bfloat16
I16 = mybir.dt.int16
AX = mybir.AxisListType
OP = mybir.AluOpType
BFNP = ml_dtypes.bfloat16


def _wrap16(idxs):
    """Lay out idx stream positions j -> [j%16, j//16], replicated across all
    eight 16-partition groups (the Q7 ucode reads group 16..31 on queue 0)."""
    n = len(idxs)
    cols = (n + 15) // 16
    t = np.zeros((16, cols), dtype=np.int16)
    t[np.arange(n) % 16, np.arange(n) // 16] = idxs
    return np.tile(t, (8, 1))


class Layout:
    pass


def _pick_classes(deg, chunk, n_nodes):
    """DP over degree boundaries minimizing total padded slots."""
    dmax = int(deg.max())
    counts = np.zeros((NCORES, dmax + 1), dtype=np.int64)
    for c in range(NCORES):
        d = deg[c * chunk:min(n_nodes, (c + 1) * chunk)]
        counts[c] = np.bincount(d, minlength=dmax + 1)
    cum = counts.cumsum(axis=1)  # cum[c, w] = nodes with deg <= w

    def cost(lo, w):  # class covers degrees (lo, w]
        n = cum[:, w] - cum[:, lo]
        cap = ((n + P - 1) // P * P).max()
        return int(cap) * w

    INF = float("inf")
    best = [0.0] + [INF] * dmax
    prev = [0] * (dmax + 1)
    for w in range(1, dmax + 1):
        for lo in range(w):
            v = best[lo] + cost(lo, w)
            if v < best[w]:
                best[w], prev[w] = v, lo
    bounds = []
    w = dmax
    while w > 0:
        bounds.append(w)
        w = prev[w]
    return sorted(bounds)


def build_layout(src, dst, n_nodes):
    src = np.asarray(src).astype(np.int64)
    dst = np.asarray(dst).astype(np.int64)
    N = n_nodes
    chunk = (N + NCORES - 1) // NCORES

    deg = np.bincount(dst, minlength=N)
    order = np.argsort(dst, kind="stable")
    src_sorted = src[order]
    starts = np.zeros(N + 1, dtype=np.int64)
    np.cumsum(deg, out=starts[1:])

    classes = _pick_classes(deg, chunk, N)

    # per-core per-class node lists + uniform caps
    node_lists = {}
    for c in range(NCORES):
        nlo, nhi = c * chunk, min(N, (c + 1) * chunk)
        d = deg[nlo:nhi]
        lo = 0
        for w in classes:
            node_lists[(c, w)] = np.nonzero((d > lo) & (d <= w))[0] + nlo
            lo = w
    caps = {}
    for w in classes:
        cap = max(len(node_lists[(c, w)]) for c in range(NCORES))
        caps[w] = ((cap + P - 1) // P) * P

    n_class_rows = sum(caps.values())
    deg0_max = max(
        int((deg[c * chunk:min(N, (c + 1) * chunk)] == 0).sum())
        for c in range(NCORES)
    )
    R = ((n_class_rows + deg0_max + P - 1) // P) * P
    T = R // P

    base = {}
    b = 0
    for w in classes:
        base[w] = b
        b += caps[w]

    perm = np.full((NCORES, R), -1, dtype=np.int64)
    row_of = np.full(N, -1, dtype=np.int64)
    for c in range(NCORES):
        nlo, nhi = c * chunk, min(N, (c + 1) * chunk)
        for w in classes:
            nl = node_lists[(c, w)]
            perm[c, base[w]:base[w] + len(nl)] = nl
            row_of[nl] = c * R + base[w] + np.arange(len(nl))
        deg0 = np.nonzero(deg[nlo:nhi] == 0)[0] + nlo
        perm[c, n_class_rows:n_class_rows + len(deg0)] = deg0
        row_of[deg0] = c * R + n_class_rows + np.arange(len(deg0))

    assert NCORES * R - LO_ROWS <= 32767, "hi gather region exceeds int16"

    # edge-slot chunks (uniform across cores): (w, nt0, t_in_class, ct)
    chunks = []
    nt = 0
    for w in classes:
        tiles = caps[w] // P
        ct0 = max(1, SLOTS_PER_CHUNK // w)
        t = 0
        while t < tiles:
            ct = min(ct0, tiles - t)
            chunks.append((w, nt, t, ct))
            nt += ct
            t += ct
    n_class_tiles = nt

    # per-core: unique src rows -> compact table; edge idx + mask tables
    uniq_lo, uniq_hi = [], []
    for c in range(NCORES):
        sel = (dst >= c * chunk) & (dst < (c + 1) * chunk)
        rows = np.unique(row_of[src[sel]])
        uniq_lo.append(rows[rows < LO_ROWS])
        uniq_hi.append(rows[rows >= LO_ROWS])
    NLO = max((len(u) + P - 1) // P * P for u in uniq_lo)
    NHI = max((len(u) + P - 1) // P * P for u in uniq_hi)
    NU = NLO + NHI
    assert NU <= 32767, f"compact table {NU} exceeds int16"

    comp_idx = np.zeros((NCORES, NLO + NHI), dtype=np.int64)
    cpos = np.zeros((NCORES, NCORES * R), dtype=np.int32)
    for c in range(NCORES):
        lo, hi = uniq_lo[c], uniq_hi[c]
        comp_idx[c, :len(lo)] = lo
        comp_idx[c, NLO:NLO + len(hi)] = hi - LO_ROWS
        cpos[c, lo] = np.arange(len(lo))
        cpos[c, hi] = NLO + np.arange(len(hi))

    # comp gather chunk list: (region_row_ofs, region_rows, n, tc_base, col_ofs)
    comp_chunks = []
    for reg_ofs, reg_rows, start, total in (
        (0, LO_ROWS, 0, NLO),
        (LO_ROWS, NCORES * R - LO_ROWS, NLO, NHI),
    ):
        done = 0
        while done < total:
            n = min(COMP_CHUNK, total - done)
            comp_chunks.append(
                (reg_ofs, reg_rows, n, start + done, (start + done) // 16)
            )
            done += n

    # edge idx + mask, chunk-slot-major: j = s*128 + p, s = t_loc*w + ws
    S_tot = sum(w * ct for (w, _, _, ct) in chunks)
    eidx = np.zeros((NCORES, 128 * S_tot), dtype=np.int64)
    emask = np.full((NCORES, P, S_tot), NEG, dtype=np.float32)
    for c in range(NCORES):
        # per-class [cap, w] idx/valid matrices, vectorized over nodes
        mats = {}
        for w in classes:
            cap = caps[w]
            nodes = perm[c, base[w]:base[w] + cap]
            nd = np.maximum(nodes, 0)
            dg = np.where(nodes >= 0, deg[nd], 0)
            im = np.zeros((cap, w), dtype=np.int64)
            vm = np.zeros((cap, w), dtype=bool)
            for ws in range(w):
                ok = ws < dg
                sidx = starts[nd] + ws
                im[ok, ws] = cpos[c, row_of[src_sorted[np.minimum(
                    sidx, len(src_sorted) - 1)]]][ok]
                vm[:, ws] = ok
            vm[nodes < 0, 0] = True  # pad row: 1 live slot, no NaN
            mats[w] = (im, vm)
        j0 = 0
        s0 = 0
        for (w, nt0, t_in_class, ct) in chunks:
            im, vm = mats[w]
            blk = slice(t_in_class * P, (t_in_class + ct) * P)
            imc = im[blk].reshape(ct, P, w)
            vmc = vm[blk].reshape(ct, P, w)
            n = ct * w * P
            eidx[c, j0:j0 + n] = imc.transpose(0, 2, 1).ravel()
            emask[c][:, s0:s0 + ct * w] = np.where(
                vmc.transpose(1, 0, 2).reshape(P, ct * w), 0.0, NEG)
            j0 += 128 * ct * w
            s0 += ct * w
    lay = Layout()
    lay.N, lay.R, lay.T, lay.chunk = N, R, T, chunk
    lay.classes, lay.caps, lay.base = classes, caps, base
    lay.n_class_tiles = n_class_tiles
    lay.chunks, lay.comp_chunks = chunks, comp_chunks
    lay.NLO, lay.NHI, lay.NU, lay.S_tot = NLO, NHI, NU, S_tot
    lay.perm, lay.row_of = perm, row_of
    lay.eidx, lay.emask, lay.comp_idx = eidx, emask, comp_idx
    return lay


def host_inputs(lay, x, qkv_w, qkv_b, out_w, out_b, out_w_last, out_b_last):
    x = np.asarray(x, dtype=np.float32)
    nclass = out_w_last.shape[1]
    wc = np.zeros((L, D, 3 * D), dtype=np.float32)
    bc = np.zeros((L, P, 3 * D), dtype=np.float32)
    wo = np.zeros((L, D, D), dtype=np.float32)
    boT = np.zeros((D, L), dtype=np.float32)
    bo_last = np.tile(out_b_last[None, :], (P, 1)).astype(np.float32)
    for l in range(L):
        wq, wk, wv = qkv_w[l, 0], qkv_w[l, 1], qkv_w[l, 2]
        bq, bk, bv = qkv_b[l, 0], qkv_b[l, 1], qkv_b[l, 2]
        wc[l] = np.concatenate([wq, wk * SCALE, wv], axis=1)
        bc[l] = np.tile(np.concatenate([bq, bk * SCALE, bv])[None, :], (P, 1))
        if l < L - 1:
            wo[l] = out_w[l]
            boT[:, l] = out_b[l]
        else:
            wo[l, :, :nclass] = out_w_last

    in_maps = []
    for c in range(NCORES):
        xp = np.where((lay.perm[c] >= 0)[:, None],
                      x[np.maximum(lay.perm[c], 0)], 0.0)
        m = {
            "x0T": np.ascontiguousarray(xp.T).astype(BFNP),
            "wc": wc.astype(BFNP), "bc": bc,
            "wo": wo.astype(BFNP), "boT": boT, "bo_last": bo_last,
            "eidx": _wrap16(lay.eidx[c]),
            "emask": lay.emask[c],
            "cidx": _wrap16(lay.comp_idx[c]),
        }
        in_maps.append(m)
    return in_maps


def host_output(lay, outs, nclass):
    full = np.zeros((lay.N, nclass), dtype=np.float32)
    for c in range(NCORES):
        real = lay.perm[c] >= 0
        full[lay.perm[c][real]] = outs[c][real]
    return full


# ----------------------------------------------------------------------------
# Device program
# ----------------------------------------------------------------------------

_PHASES = ["ABCD"]  # debug: phase bisect knob


def build_nc(lay, nclass):
    R, T = lay.R, lay.T
    nc = bacc.Bacc(trn_type="TRN2", num_devices=NCORES)

    x0T = nc.dram_tensor("x0T", [D, R], BF, kind="ExternalInput")
    wc = nc.dram_tensor("wc", [L, D, 3 * D], BF, kind="ExternalInput")
    bc = nc.dram_tensor("bc", [L, P, 3 * D], FP, kind="ExternalInput")
    wo = nc.dram_tensor("wo", [L, D, D], BF, kind="ExternalInput")
    boT = nc.dram_tensor("boT", [D, L], FP, kind="ExternalInput")
    bo_last = nc.dram_tensor("bo_last", [P, nclass], FP, kind="ExternalInput")
    eidx_d = nc.dram_tensor(
        "eidx", [128, (128 * lay.S_tot) // 16], I16, kind="ExternalInput")
    emask_d = nc.dram_tensor("emask", [P, lay.S_tot], FP, kind="ExternalInput")
    cidx_d = nc.dram_tensor(
        "cidx", [128, lay.NU // 16], I16, kind="ExternalInput")
    out_ext = nc.dram_tensor("out", [R, nclass], FP, kind="ExternalOutput")

    qv_slice = nc.dram_tensor("qv_slice", [R, 2 * D], BF, kind="Internal")
    qv_full = nc.dram_tensor(
        "qv_full", [NCORES * R, 2 * D], BF, kind="Internal", addr_space="Shared"
    )
    tcomp = nc.dram_tensor("tcomp", [lay.NU, 2 * D], BF, kind="Internal")
    rg = [list(range(NCORES))]

    CT_MAX = max(ct for (_, _, _, ct) in lay.chunks)
    with tile.TileContext(nc) as tc:
        with (
            tc.tile_pool(name="const", bufs=1) as cpool,
            tc.tile_pool(name="persist", bufs=1) as ppool,
            tc.tile_pool(name="proj", bufs=4) as projpool,
            tc.tile_pool(name="work", bufs=2) as wpool,
            tc.tile_pool(name="small", bufs=3) as spool,
            tc.tile_pool(name="psum", bufs=2, space="PSUM") as pspool,
            tc.tile_pool(name="psum_o", bufs=2, space="PSUM") as pspool_o,
        ):
            ident = cpool.tile([P, P], BF, tag="ident", name="ident")
            make_identity(nc, ident[:])
            wc_sb = cpool.tile([P, L * 3 * D], BF, tag="wc", name="wc")
            nc.sync.dma_start(
                wc_sb[:].rearrange("k (l n) -> k l n", l=L),
                wc[:].rearrange("l k n -> k l n"))
            bc_sb = cpool.tile([P, L * 3 * D], FP, tag="bc", name="bc")
            nc.sync.dma_start(
                bc_sb[:].rearrange("p (l n) -> p l n", l=L),
                bc[:].rearrange("l p n -> p l n"))
            wo_sb = cpool.tile([P, L * D], BF, tag="wo", name="wo")
            nc.sync.dma_start(
                wo_sb[:].rearrange("k (l n) -> k l n", l=L),
                wo[:].rearrange("l k n -> k l n"))
            boT_sb = cpool.tile([P, L], FP, tag="boT", name="boT")
            nc.sync.dma_start(boT_sb[:], boT[:])
            bol_sb = cpool.tile([P, nclass], FP, tag="bol", name="bol")
            nc.sync.dma_start(bol_sb[:], bo_last[:])
            eidx_sb = cpool.tile(
                [128, (128 * lay.S_tot) // 16], I16, tag="eidx", name="eidx")
            nc.sync.dma_start(eidx_sb[:], eidx_d[:])
            mask_sb = cpool.tile([P, lay.S_tot], FP, tag="mask", name="mask")
            nc.sync.dma_start(mask_sb[:], emask_d[:])
            cidx_sb = cpool.tile([128, lay.NU // 16], I16, tag="cidx", name="cidx")
            nc.sync.dma_start(cidx_sb[:], cidx_d[:])

            xT_sb = ppool.tile([P, R], BF, tag="x", name="x")
            nc.sync.dma_start(xT_sb[:], x0T[:])
            k_sb = ppool.tile([P, T * D], BF, tag="k", name="k")
            tc.strict_bb_all_engine_barrier()

            for l in range(L):
                # ---- Phase A: QKV projection (x kept feature-major) ----
                for t in range(T):
                    qkv_ps = pspool.tile([P, 3 * D], FP, tag="qkv", name="qkv")
                    nc.tensor.matmul(
                        qkv_ps[:], lhsT=xT_sb[:, t * P:(t + 1) * P],
                        rhs=wc_sb[:, l * 3 * D:(l + 1) * 3 * D],
                        start=True, stop=True,
                    )
                    bofs = l * 3 * D
                    qv_st = projpool.tile([P, 2 * D], BF, tag="qv_st", name="qv_st")
                    nc.vector.tensor_tensor(
                        out=qv_st[:, 0:D], in0=qkv_ps[:, 0:D],
                        in1=bc_sb[:, bofs:bofs + D], op=OP.add)
                    nc.vector.tensor_tensor(
                        out=qv_st[:, D:2 * D], in0=qkv_ps[:, 2 * D:3 * D],
                        in1=bc_sb[:, bofs + 2 * D:bofs + 3 * D], op=OP.add)
                    nc.vector.tensor_tensor(
                        out=k_sb[:, t * D:(t + 1) * D], in0=qkv_ps[:, D:2 * D],
                        in1=bc_sb[:, bofs + D:bofs + 2 * D], op=OP.add)
                    nc.sync.dma_start(qv_slice[t * P:(t + 1) * P, :], qv_st[:])

                # ---- Phase B: exchange qv ----
                nc.gpsimd.collective_compute(
                    "AllGather", OP.bypass, replica_groups=rg,
                    ins=[qv_slice[:, :]], outs=[qv_full[:, :]],
                )
                tc.strict_bb_all_engine_barrier()

                # ---- Phase C: compact the unique src rows ----
                for (reg_ofs, reg_rows, n, tc_base, col_ofs) in (
                        lay.comp_chunks if "C" in _PHASES[0] else []):
                    cg = wpool.tile(
                        [P, COMP_CHUNK // P * 2 * D], BF, tag="cg", name="cg")
                    sl = n // P * 2 * D
                    nc.gpsimd.dma_gather(
                        cg[:, 0:sl].rearrange("p (s d) -> p s d", d=2 * D),
                        qv_full[reg_ofs:reg_ofs + reg_rows, :],
                        cidx_sb[:, col_ofs:col_ofs + n // 16],
                        n, n, 2 * D,
                    )
                    nc.sync.dma_start(
                        tcomp[tc_base:tc_base + n, :].rearrange(
                            "(s p) d -> p s d", p=P),
                        cg[:, 0:sl].rearrange("p (s d) -> p s d", d=2 * D),
                    )

                # ---- Phase D: edge softmax + aggregation + out projection ----
                ecol = 0
                scol = 0
                for (w, nt0, t_in_class, ct) in (
                        lay.chunks if "D" in _PHASES[0] else []):
                    S = ct * w
                    qv_g = wpool.tile(
                        [P, SLOTS_PER_CHUNK * 2 * D], BF, tag="qvg", name="qvg")
                    off = 0
                    while off < S:
                        gs = min(GATHER_MAX_SLOTS, S - off)
                        nc.gpsimd.dma_gather(
                            qv_g[:, off * 2 * D:(off + gs) * 2 * D]
                                .rearrange("p (s d) -> p s d", d=2 * D),
                            tcomp[:, :],
                            eidx_sb[:, ecol + off * 8:ecol + (off + gs) * 8],
                            128 * gs, 128 * gs, 2 * D,
                        )
                        off += gs
                    # scores: per-head dot(q_gathered, k_local)
                    tmp = wpool.tile(
                        [P, SLOTS_PER_CHUNK * D], BF, tag="tmp", name="tmp")
                    q4 = qv_g[:, 0:S * 2 * D].rearrange(
                        "p (t w c) -> p t w c", w=w, c=2 * D)[:, :, :, 0:D]
                    k4 = (k_sb[:, nt0 * D:(nt0 + ct) * D]
                          .rearrange("p (t d) -> p t d", d=D)
                          .unsqueeze(2).to_broadcast([P, ct, w, D]))
                    nc.vector.tensor_tensor(
                        out=tmp[:, 0:S * D].rearrange(
                            "p (t w d) -> p t w d", w=w, d=D),
                        in0=q4, in1=k4, op=OP.mult)
                    s_t = spool.tile([P, SLOTS_PER_CHUNK * H], FP, tag="s", name="s")
                    nc.vector.reduce_sum(
                        s_t[:, 0:S * H].rearrange("p (s h) -> p s h", h=H),
                        tmp[:, 0:S * D].rearrange(
                            "p (s h e) -> p s h e", h=H, e=HD),
                        axis=AX.X)
                    nc.vector.tensor_tensor(
                        out=s_t[:, 0:S * H].rearrange("p (s h) -> p s h", h=H),
                        in0=s_t[:, 0:S * H].rearrange("p (s h) -> p s h", h=H),
                        in1=mask_sb[:, scol:scol + S]
                            .unsqueeze(2).to_broadcast([P, S, H]),
                        op=OP.add)
                    ex = spool.tile([P, SLOTS_PER_CHUNK * H], BF, tag="ex", name="ex")
                    nc.scalar.activation(
                        out=ex[:, 0:S * H], in_=s_t[:, 0:S * H],
                        func=mybir.ActivationFunctionType.Exp)
                    denom = spool.tile([P, CT_MAX * H], FP, tag="dn", name="dn")
                    nc.vector.reduce_sum(
                        denom[:, 0:ct * H].rearrange("p (t h) -> p t h", h=H),
                        ex[:, 0:S * H].rearrange(
                            "p (t w h) -> p t h w", w=w, h=H),
                        axis=AX.X)
                    rec = spool.tile([P, CT_MAX * H], FP, tag="rc", name="rc")
                    nc.vector.reciprocal(rec[:, 0:ct * H], denom[:, 0:ct * H])
                    # weighted v
                    nc.vector.tensor_tensor(
                        out=tmp[:, 0:S * D].rearrange(
                            "p (s h e) -> p s h e", h=H, e=HD),
                        in0=qv_g[:, 0:S * 2 * D].rearrange(
                            "p (s c) -> p s c", c=2 * D)[:, :, D:2 * D]
                            .rearrange("p s (h e) -> p s h e", h=H),
                        in1=ex[:, 0:S * H].rearrange("p (s h) -> p s h", h=H)
                            .unsqueeze(3).to_broadcast([P, S, H, HD]),
                        op=OP.mult)
                    agg = spool.tile([P, CT_MAX * D], FP, tag="agg", name="agg")
                    nc.vector.reduce_sum(
                        agg[:, 0:ct * D].rearrange("p (t d) -> p t d", d=D),
                        tmp[:, 0:S * D].rearrange(
                            "p (t w d) -> p t d w", w=w, d=D),
                        axis=AX.X)
                    aggn = spool.tile([P, CT_MAX * D], BF, tag="aggn", name="aggn")
                    nc.vector.tensor_tensor(
                        out=aggn[:, 0:ct * D].rearrange(
                            "p (t h e) -> p t h e", h=H, e=HD),
                        in0=agg[:, 0:ct * D].rearrange(
                            "p (t h e) -> p t h e", h=H, e=HD),
                        in1=rec[:, 0:ct * H].rearrange("p (t h) -> p t h", h=H)
                            .unsqueeze(3).to_broadcast([P, ct, H, HD]),
                        op=OP.mult)
                    # fused output projection, one node tile at a time
                    for tl in range(ct):
                        nt = nt0 + tl
                        aT_ps = pspool_o.tile([P, P], BF, tag="aT", name="aT")
                        nc.tensor.transpose(
                            aT_ps[:], aggn[:, tl * D:(tl + 1) * D], ident[:])
                        aT = projpool.tile([P, P], BF, tag="aT_sb", name="aT_sb")
                        nc.vector.tensor_copy(aT[:], aT_ps[:])
                        if l < L - 1:
                            oT_ps = pspool_o.tile([P, P], FP, tag="oT", name="oT")
                            nc.tensor.matmul(
                                oT_ps[:], lhsT=wo_sb[:, l * D:(l + 1) * D],
                                rhs=aT[:], start=True, stop=True)
                            nc.vector.tensor_tensor(
                                out=xT_sb[:, nt * P:(nt + 1) * P],
                                in0=oT_ps[:],
                                in1=boT_sb[:, l:l + 1].to_broadcast([P, P]),
                                op=OP.add)
                        else:
                            o_ps = pspool_o.tile([P, nclass], FP, tag="o", name="o")
                            nc.tensor.matmul(
                                o_ps[:], lhsT=aT[:],
                                rhs=wo_sb[:, l * D:l * D + nclass],
                                start=True, stop=True)
                            o_sb = projpool.tile(
                                [P, nclass], FP, tag="o_sb", name="o_sb")
                            nc.vector.tensor_tensor(
                                out=o_sb[:], in0=o_ps[:], in1=bol_sb[:],
                                op=OP.add)
                            nc.sync.dma_start(
                                out_ext[nt * P:(nt + 1) * P, :], o_sb[:])
                    ecol += (128 * S) // 16
                    scol += S
                # tail tiles (deg-0 + padding rows): out = bias only
                for t in range(lay.n_class_tiles, T):
                    if l < L - 1:
                        nc.vector.tensor_copy(
                            xT_sb[:, t * P:(t + 1) * P],
                            boT_sb[:, l:l + 1].to_broadcast([P, P]))
                    else:
                        o_sb = projpool.tile(
                            [P, nclass], FP, tag="o_sb", name="o_sb")
                        nc.vector.tensor_copy(o_sb[:], bol_sb[:])
                        nc.sync.dma_start(out_ext[t * P:(t + 1) * P, :], o_sb[:])
    nc.compile()
    return nc


# ----------------------------------------------------------------------------
# Entry point
# ----------------------------------------------------------------------------

_trace = [False]  # test.py can flip this to profile


def kernel(x, src, dst, qkv_w, qkv_b, out_w, out_b, out_w_last, out_b_last):
    x = np.asarray(x, dtype=np.float32)
    lay = build_layout(np.asarray(src), np.asarray(dst), x.shape[0])
    nclass = np.asarray(out_w_last).shape[1]
    in_maps = host_inputs(
        lay, x, np.asarray(qkv_w, dtype=np.float32),
        np.asarray(qkv_b, dtype=np.float32), np.asarray(out_w, dtype=np.float32),
        np.asarray(out_b, dtype=np.float32),
        np.asarray(out_w_last, dtype=np.float32),
        np.asarray(out_b_last, dtype=np.float32),
    )
    nc = build_nc(lay, nclass)
    res = run_bass_kernel_spmd(
        nc, in_maps, core_ids=list(range(NCORES)), trace=_trace[0]
    )
    kernel.last_results = res
    outs = [np.asarray(res.results[c]["out"], dtype=np.float32)
            for c in range(NCORES)]
    return host_output(lay, outs, nclass)


# revision 10
# speedup vs baseline: 1.4162x; 1.2738x over previous
"""Graph-Transformer message-passing kernel for 8 Trainium2 NeuronCores.

Strategy (v2, dst-shard + batched SWDGE gathers, bf16):
  - Nodes split into 8 contiguous dst ranges; core c owns all edges into its
    range, so softmax/aggregation are fully local to a core.
  - Host groups each core's nodes into degree classes (DP-optimized widths),
    pads each node's in-edge list to the class width W; pad slots carry a
    -1e30 mask so they vanish in the softmax.
  - Per layer: each core projects Q|K|V for its rows (PE, bf16), the q|v
    halves are AllGathered into a global [8R, 256] bf16 table, each core then
    compacts the ~31.7K unique source rows it needs into a private <32768-row
    table (two dma_gather passes, int16 idx limit), and edge-slot q|v rows are
    batch-gathered from it (one dma_gather per ~32-slot chunk instead of one
    indirect DMA per edge slot: ~1us fixed SWDGE cost amortized 32x).
  - Masked softmax (no max-subtraction: |score| < 4 for this model family) and
    weighted aggregation run on DVE in bf16; output projection on PE keeps x
    feature-major (xT) so the next layer's QKV needs no transpose.
  - 3 layers fused in one NEFF; host inverts the node permutation.
"""

import numpy as np
import ml_dtypes

import concourse.bass as bass
import concourse.bacc as bacc
import concourse.mybir as mybir
import concourse.tile as tile
from concourse.masks import make_identity
from concourse.bass_utils import run_bass_kernel_spmd

NCORES = 8
L = 3
H = 8
D = 128
HD = D // H
SCALE = 1.0 / float(np.sqrt(HD))
NEG = -1.0e30
P = 128
LO_ROWS = 32768          # int16 idx limit for the low gather region
SLOTS_PER_CHUNK = 32     # edge-slot columns per dma_gather / DVE chunk
COMP_CHUNK = 1024        # rows per compaction dma_gather (SWDGE ring: <=65 descs/ring)
GATHER_MAX_SLOTS = 8     # 128*8 = 1024 idxs per edge dma_gather

FP = mybir.dt.float32
BF = mybir.dt.bfloat16
I16 = mybir.dt.int16
AX = mybir.AxisListType
OP = mybir.AluOpType
BFNP = ml_dtypes.bfloat16


def _wrap16(idxs):
    """Lay out idx stream positions j -> [j%16, j//16], replicated across all
    eight 16-partition groups (the Q7 ucode reads group 16..31 on queue 0)."""
    n = len(idxs)
    cols = (n + 15) // 16
    t = np.zeros((16, cols), dtype=np.int16)
    t[np.arange(n) % 16, np.arange(n) // 16] = idxs
    return np.tile(t, (8, 1))


class Layout:
    pass


def _pick_classes(deg, chunk, n_nodes):
    """DP over degree boundaries minimizing total padded slots."""
    dmax = int(deg.max())
    counts = np.zeros((NCORES, dmax + 1), dtype=np.int64)
    for c in range(NCORES):
        d = deg[c * chunk:min(n_nodes, (c + 1) * chunk)]
        counts[c] = np.bincount(d, minlength=dmax + 1)
    cum = counts.cumsum(axis=1)  # cum[c, w] = nodes with deg <= w

    def cost(lo, w):  # class covers degrees (lo, w]
        n = cum[:, w] - cum[:, lo]
        cap = ((n + P - 1) // P * P).max()
        return int(cap) * w

    INF = float("inf")
    best = [0.0] + [INF] * dmax
    prev = [0] * (dmax + 1)
    for w in range(1, dmax + 1):
        for lo in range(w):
            v = best[lo] + cost(lo, w)
            if v < best[w]:
                best[w], prev[w] = v, lo
    bounds = []
    w = dmax
    while w > 0:
        bounds.append(w)
        w = prev[w]
    return sorted(bounds)


def build_layout(src, dst, n_nodes):
    src = np.asarray(src).astype(np.int64)
    dst = np.asarray(dst).astype(np.int64)
    N = n_nodes
    chunk = (N + NCORES - 1) // NCORES

    deg = np.bincount(dst, minlength=N)
    order = np.argsort(dst, kind="stable")
    src_sorted = src[order]
    starts = np.zeros(N + 1, dtype=np.int64)
    np.cumsum(deg, out=starts[1:])

    classes = _pick_classes(deg, chunk, N)

    # per-core per-class node lists + uniform caps
    node_lists = {}
    for c in range(NCORES):
        nlo, nhi = c * chunk, min(N, (c + 1) * chunk)
        d = deg[nlo:nhi]
        lo = 0
        for w in classes:
            node_lists[(c, w)] = np.nonzero((d > lo) & (d <= w))[0] + nlo
            lo = w
    caps = {}
    for w in classes:
        cap = max(len(node_lists[(c, w)]) for c in range(NCORES))
        caps[w] = ((cap + P - 1) // P) * P

    n_class_rows = sum(caps.values())
    deg0_max = max(
        int((deg[c * chunk:min(N, (c + 1) * chunk)] == 0).sum())
        for c in range(NCORES)
    )
    R = ((n_class_rows + deg0_max + P - 1) // P) * P
    T = R // P

    base = {}
    b = 0
    for w in classes:
        base[w] = b
        b += caps[w]

    perm = np.full((NCORES, R), -1, dtype=np.int64)
    row_of = np.full(N, -1, dtype=np.int64)
    for c in range(NCORES):
        nlo, nhi = c * chunk, min(N, (c + 1) * chunk)
        for w in classes:
            nl = node_lists[(c, w)]
            perm[c, base[w]:base[w] + len(nl)] = nl
            row_of[nl] = c * R + base[w] + np.arange(len(nl))
        deg0 = np.nonzero(deg[nlo:nhi] == 0)[0] + nlo
        perm[c, n_class_rows:n_class_rows + len(deg0)] = deg0
        row_of[deg0] = c * R + n_class_rows + np.arange(len(deg0))

    assert NCORES * R - LO_ROWS <= 32767, "hi gather region exceeds int16"

    # edge-slot chunks (uniform across cores): (w, nt0, t_in_class, ct)
    chunks = []
    nt = 0
    for w in classes:
        tiles = caps[w] // P
        ct0 = max(1, SLOTS_PER_CHUNK // w)
        t = 0
        while t < tiles:
            ct = min(ct0, tiles - t)
            chunks.append((w, nt, t, ct))
            nt += ct
            t += ct
    n_class_tiles = nt

    # per-core: unique src rows -> compact table; edge idx + mask tables
    uniq_lo, uniq_hi = [], []
    for c in range(NCORES):
        sel = (dst >= c * chunk) & (dst < (c + 1) * chunk)
        rows = np.unique(row_of[src[sel]])
        uniq_lo.append(rows[rows < LO_ROWS])
        uniq_hi.append(rows[rows >= LO_ROWS])
    NLO = max((len(u) + P - 1) // P * P for u in uniq_lo)
    NHI = max((len(u) + P - 1) // P * P for u in uniq_hi)
    NU = NLO + NHI
    assert NU <= 32767, f"compact table {NU} exceeds int16"

    comp_idx = np.zeros((NCORES, NLO + NHI), dtype=np.int64)
    cpos = np.zeros((NCORES, NCORES * R), dtype=np.int32)
    for c in range(NCORES):
        lo, hi = uniq_lo[c], uniq_hi[c]
        comp_idx[c, :len(lo)] = lo
        comp_idx[c, NLO:NLO + len(hi)] = hi - LO_ROWS
        cpos[c, lo] = np.arange(len(lo))
        cpos[c, hi] = NLO + np.arange(len(hi))

    # comp gather chunk list: (region_row_ofs, region_rows, n, tc_base, col_ofs)
    comp_chunks = []
    for reg_ofs, reg_rows, start, total in (
        (0, LO_ROWS, 0, NLO),
        (LO_ROWS, NCORES * R - LO_ROWS, NLO, NHI),
    ):
        done = 0
        while done < total:
            n = min(COMP_CHUNK, total - done)
            comp_chunks.append(
                (reg_ofs, reg_rows, n, start + done, (start + done) // 16)
            )
            done += n

    # edge idx + mask, chunk-slot-major: j = s*128 + p, s = t_loc*w + ws
    S_tot = sum(w * ct for (w, _, _, ct) in chunks)
    eidx = np.zeros((NCORES, 128 * S_tot), dtype=np.int64)
    emask = np.full((NCORES, P, S_tot), NEG, dtype=np.float32)
    for c in range(NCORES):
        # per-class [cap, w] idx/valid matrices, vectorized over nodes
        mats = {}
        for w in classes:
            cap = caps[w]
            nodes = perm[c, base[w]:base[w] + cap]
            nd = np.maximum(nodes, 0)
            dg = np.where(nodes >= 0, deg[nd], 0)
            im = np.zeros((cap, w), dtype=np.int64)
            vm = np.zeros((cap, w), dtype=bool)
            for ws in range(w):
                ok = ws < dg
                sidx = starts[nd] + ws
                im[ok, ws] = cpos[c, row_of[src_sorted[np.minimum(
                    sidx, len(src_sorted) - 1)]]][ok]
                vm[:, ws] = ok
            vm[nodes < 0, 0] = True  # pad row: 1 live slot, no NaN
            mats[w] = (im, vm)
        j0 = 0
        s0 = 0
        for (w, nt0, t_in_class, ct) in chunks:
            im, vm = mats[w]
            blk = slice(t_in_class * P, (t_in_class + ct) * P)
            imc = im[blk].reshape(ct, P, w)
            vmc = vm[blk].reshape(ct, P, w)
            n = ct * w * P
            eidx[c, j0:j0 + n] = imc.transpose(0, 2, 1).ravel()
            emask[c][:, s0:s0 + ct * w] = np.where(
                vmc.transpose(1, 0, 2).reshape(P, ct * w), 0.0, NEG)
            j0 += 128 * ct * w
            s0 += ct * w
    lay = Layout()
    lay.N, lay.R, lay.T, lay.chunk = N, R, T, chunk
    lay.classes, lay.caps, lay.base = classes, caps, base
    lay.n_class_tiles = n_class_tiles
    lay.chunks, lay.comp_chunks = chunks, comp_chunks
    lay.NLO, lay.NHI, lay.NU, lay.S_tot = NLO, NHI, NU, S_tot
    lay.perm, lay.row_of = perm, row_of
    lay.eidx, lay.emask, lay.comp_idx = eidx, emask, comp_idx
    return lay


def host_inputs(lay, x, qkv_w, qkv_b, out_w, out_b, out_w_last, out_b_last):
    x = np.asarray(x, dtype=np.float32)
    nclass = out_w_last.shape[1]
    wc = np.zeros((L, D, 3 * D), dtype=np.float32)
    bc = np.zeros((L, P, 3 * D), dtype=np.float32)
    wo = np.zeros((L, D, D), dtype=np.float32)
    boT = np.zeros((D, L), dtype=np.float32)
    bo_last = np.tile(out_b_last[None, :], (P, 1)).astype(np.float32)
    for l in range(L):
        wq, wk, wv = qkv_w[l, 0], qkv_w[l, 1], qkv_w[l, 2]
        bq, bk, bv = qkv_b[l, 0], qkv_b[l, 1], qkv_b[l, 2]
        wc[l] = np.concatenate([wq, wk * SCALE, wv], axis=1)
        bc[l] = np.tile(np.concatenate([bq, bk * SCALE, bv])[None, :], (P, 1))
        if l < L - 1:
            wo[l] = out_w[l]
            boT[:, l] = out_b[l]
        else:
            wo[l, :, :nclass] = out_w_last

    in_maps = []
    for c in range(NCORES):
        xp = np.where((lay.perm[c] >= 0)[:, None],
                      x[np.maximum(lay.perm[c], 0)], 0.0)
        m = {
            "x0T": np.ascontiguousarray(xp.T).astype(BFNP),
            "wc": wc.astype(BFNP), "bc": bc,
            "wo": wo.astype(BFNP), "boT": boT, "bo_last": bo_last,
            "eidx": _wrap16(lay.eidx[c]),
            "emask": lay.emask[c],
            "cidx": _wrap16(lay.comp_idx[c]),
        }
        in_maps.append(m)
    return in_maps


def host_output(lay, outs, nclass):
    full = np.zeros((lay.N, nclass), dtype=np.float32)
    for c in range(NCORES):
        real = lay.perm[c] >= 0
        full[lay.perm[c][real]] = outs[c][real]
    return full


# ----------------------------------------------------------------------------
# Device program
# ----------------------------------------------------------------------------

_PHASES = ["ABCD"]  # debug: phase bisect knob


def build_nc(lay, nclass):
    R, T = lay.R, lay.T
    nc = bacc.Bacc(trn_type="TRN2", num_devices=NCORES, num_swdge_queues=4)

    x0T = nc.dram_tensor("x0T", [D, R], BF, kind="ExternalInput")
    wc = nc.dram_tensor("wc", [L, D, 3 * D], BF, kind="ExternalInput")
    bc = nc.dram_tensor("bc", [L, P, 3 * D], FP, kind="ExternalInput")
    wo = nc.dram_tensor("wo", [L, D, D], BF, kind="ExternalInput")
    boT = nc.dram_tensor("boT", [D, L], FP, kind="ExternalInput")
    bo_last = nc.dram_tensor("bo_last", [P, nclass], FP, kind="ExternalInput")
    eidx_d = nc.dram_tensor(
        "eidx", [128, (128 * lay.S_tot) // 16], I16, kind="ExternalInput")
    emask_d = nc.dram_tensor("emask", [P, lay.S_tot], FP, kind="ExternalInput")
    cidx_d = nc.dram_tensor(
        "cidx", [128, lay.NU // 16], I16, kind="ExternalInput")
    out_ext = nc.dram_tensor("out", [R, nclass], FP, kind="ExternalOutput")

    qv_slice = nc.dram_tensor("qv_slice", [R, 2 * D], BF, kind="Internal")
    qv_full = nc.dram_tensor(
        "qv_full", [NCORES * R, 2 * D], BF, kind="Internal", addr_space="Shared"
    )
    tcomp = nc.dram_tensor("tcomp", [lay.NU, 2 * D], BF, kind="Internal")
    rg = [list(range(NCORES))]

    CT_MAX = max(ct for (_, _, _, ct) in lay.chunks)
    with tile.TileContext(nc) as tc:
        with (
            tc.tile_pool(name="const", bufs=1) as cpool,
            tc.tile_pool(name="persist", bufs=1) as ppool,
            tc.tile_pool(name="proj", bufs=4) as projpool,
            tc.tile_pool(name="work", bufs=2) as wpool,
            tc.tile_pool(name="small", bufs=3) as spool,
            tc.tile_pool(name="psum", bufs=2, space="PSUM") as pspool,
            tc.tile_pool(name="psum_o", bufs=2, space="PSUM") as pspool_o,
        ):
            ident = cpool.tile([P, P], BF, tag="ident", name="ident")
            make_identity(nc, ident[:])
            wc_sb = cpool.tile([P, L * 3 * D], BF, tag="wc", name="wc")
            nc.sync.dma_start(
                wc_sb[:].rearrange("k (l n) -> k l n", l=L),
                wc[:].rearrange("l k n -> k l n"))
            bc_sb = cpool.tile([P, L * 3 * D], FP, tag="bc", name="bc")
            nc.sync.dma_start(
                bc_sb[:].rearrange("p (l n) -> p l n", l=L),
                bc[:].rearrange("l p n -> p l n"))
            wo_sb = cpool.tile([P, L * D], BF, tag="wo", name="wo")
            nc.sync.dma_start(
                wo_sb[:].rearrange("k (l n) -> k l n", l=L),
                wo[:].rearrange("l k n -> k l n"))
            boT_sb = cpool.tile([P, L], FP, tag="boT", name="boT")
            nc.sync.dma_start(boT_sb[:], boT[:])
            bol_sb = cpool.tile([P, nclass], FP, tag="bol", name="bol")
            nc.sync.dma_start(bol_sb[:], bo_last[:])
            eidx_sb = cpool.tile(
                [128, (128 * lay.S_tot) // 16], I16, tag="eidx", name="eidx")
            nc.sync.dma_start(eidx_sb[:], eidx_d[:])
            mask_sb = cpool.tile([P, lay.S_tot], FP, tag="mask", name="mask")
            nc.sync.dma_start(mask_sb[:], emask_d[:])
            cidx_sb = cpool.tile([128, lay.NU // 16], I16, tag="cidx", name="cidx")
            nc.sync.dma_start(cidx_sb[:], cidx_d[:])

            qrr = [0]  # SWDGE queue round-robin

            def next_q():
                qrr[0] = (qrr[0] + 1) % 4
                return qrr[0]

            xT_sb = ppool.tile([P, R], BF, tag="x", name="x")
            nc.sync.dma_start(xT_sb[:], x0T[:])
            k_sb = ppool.tile([P, T * D], BF, tag="k", name="k")
            tc.strict_bb_all_engine_barrier()

            for l in range(L):
                # ---- Phase A: QKV projection (x kept feature-major) ----
                for t in range(T):
                    qkv_ps = pspool.tile([P, 3 * D], FP, tag="qkv", name="qkv")
                    nc.tensor.matmul(
                        qkv_ps[:], lhsT=xT_sb[:, t * P:(t + 1) * P],
                        rhs=wc_sb[:, l * 3 * D:(l + 1) * 3 * D],
                        start=True, stop=True,
                    )
                    bofs = l * 3 * D
                    qv_st = projpool.tile([P, 2 * D], BF, tag="qv_st", name="qv_st")
                    nc.vector.tensor_tensor(
                        out=qv_st[:, 0:D], in0=qkv_ps[:, 0:D],
                        in1=bc_sb[:, bofs:bofs + D], op=OP.add)
                    nc.vector.tensor_tensor(
                        out=qv_st[:, D:2 * D], in0=qkv_ps[:, 2 * D:3 * D],
                        in1=bc_sb[:, bofs + 2 * D:bofs + 3 * D], op=OP.add)
                    nc.vector.tensor_tensor(
                        out=k_sb[:, t * D:(t + 1) * D], in0=qkv_ps[:, D:2 * D],
                        in1=bc_sb[:, bofs + D:bofs + 2 * D], op=OP.add)
                    nc.sync.dma_start(qv_slice[t * P:(t + 1) * P, :], qv_st[:])

                # ---- Phase B: exchange qv ----
                nc.gpsimd.collective_compute(
                    "AllGather", OP.bypass, replica_groups=rg,
                    ins=[qv_slice[:, :]], outs=[qv_full[:, :]],
                )
                tc.strict_bb_all_engine_barrier()

                # ---- Phase C: compact the unique src rows ----
                for (reg_ofs, reg_rows, n, tc_base, col_ofs) in (
                        lay.comp_chunks if "C" in _PHASES[0] else []):
                    cg = wpool.tile(
                        [P, COMP_CHUNK // P * 2 * D], BF, tag="cg", name="cg")
                    sl = n // P * 2 * D
                    nc.gpsimd.dma_gather(
                        cg[:, 0:sl].rearrange("p (s d) -> p s d", d=2 * D),
                        qv_full[reg_ofs:reg_ofs + reg_rows, :],
                        cidx_sb[:, col_ofs:col_ofs + n // 16],
                        n, n, 2 * D, queue_num=next_q(),
                    )
                    nc.sync.dma_start(
                        tcomp[tc_base:tc_base + n, :].rearrange(
                            "(s p) d -> p s d", p=P),
                        cg[:, 0:sl].rearrange("p (s d) -> p s d", d=2 * D),
                    )

                # ---- Phase D: edge softmax + aggregation + out projection ----
                ecol = 0
                scol = 0
                for (w, nt0, t_in_class, ct) in (
                        lay.chunks if "D" in _PHASES[0] else []):
                    S = ct * w
                    qv_g = wpool.tile(
                        [P, SLOTS_PER_CHUNK * 2 * D], BF, tag="qvg", name="qvg")
                    off = 0
                    while off < S:
                        gs = min(GATHER_MAX_SLOTS, S - off)
                        nc.gpsimd.dma_gather(
                            qv_g[:, off * 2 * D:(off + gs) * 2 * D]
                                .rearrange("p (s d) -> p s d", d=2 * D),
                            tcomp[:, :],
                            eidx_sb[:, ecol + off * 8:ecol + (off + gs) * 8],
                            128 * gs, 128 * gs, 2 * D, queue_num=next_q(),
                        )
                        off += gs
                    # scores: per-head dot(q_gathered, k_local)
                    tmp = wpool.tile(
                        [P, SLOTS_PER_CHUNK * D], BF, tag="tmp", name="tmp")
                    q4 = qv_g[:, 0:S * 2 * D].rearrange(
                        "p (t w c) -> p t w c", w=w, c=2 * D)[:, :, :, 0:D]
                    k4 = (k_sb[:, nt0 * D:(nt0 + ct) * D]
                          .rearrange("p (t d) -> p t d", d=D)
                          .unsqueeze(2).to_broadcast([P, ct, w, D]))
                    nc.vector.tensor_tensor(
                        out=tmp[:, 0:S * D].rearrange(
                            "p (t w d) -> p t w d", w=w, d=D),
                        in0=q4, in1=k4, op=OP.mult)
                    s_t = spool.tile([P, SLOTS_PER_CHUNK * H], FP, tag="s", name="s")
                    nc.vector.reduce_sum(
                        s_t[:, 0:S * H].rearrange("p (s h) -> p s h", h=H),
                        tmp[:, 0:S * D].rearrange(
                            "p (s h e) -> p s h e", h=H, e=HD),
                        axis=AX.X)
                    nc.vector.tensor_tensor(
                        out=s_t[:, 0:S * H].rearrange("p (s h) -> p s h", h=H),
                        in0=s_t[:, 0:S * H].rearrange("p (s h) -> p s h", h=H),
                        in1=mask_sb[:, scol:scol + S]
                            .unsqueeze(2).to_broadcast([P, S, H]),
                        op=OP.add)
                    ex = spool.tile([P, SLOTS_PER_CHUNK * H], BF, tag="ex", name="ex")
                    nc.scalar.activation(
                        out=ex[:, 0:S * H], in_=s_t[:, 0:S * H],
                        func=mybir.ActivationFunctionType.Exp)
                    denom = spool.tile([P, CT_MAX * H], FP, tag="dn", name="dn")
                    nc.vector.reduce_sum(
                        denom[:, 0:ct * H].rearrange("p (t h) -> p t h", h=H),
                        ex[:, 0:S * H].rearrange(
                            "p (t w h) -> p t h w", w=w, h=H),
                        axis=AX.X)
                    rec = spool.tile([P, CT_MAX * H], FP, tag="rc", name="rc")
                    nc.vector.reciprocal(rec[:, 0:ct * H], denom[:, 0:ct * H])
                    # weighted v
                    nc.vector.tensor_tensor(
                        out=tmp[:, 0:S * D].rearrange(
                            "p (s h e) -> p s h e", h=H, e=HD),
                        in0=qv_g[:, 0:S * 2 * D].rearrange(
                            "p (s c) -> p s c", c=2 * D)[:, :, D:2 * D]
                            .rearrange("p s (h e) -> p s h e", h=H),
                        in1=ex[:, 0:S * H].rearrange("p (s h) -> p s h", h=H)
                            .unsqueeze(3).to_broadcast([P, S, H, HD]),
                        op=OP.mult)
                    agg = spool.tile([P, CT_MAX * D], FP, tag="agg", name="agg")
                    nc.vector.reduce_sum(
                        agg[:, 0:ct * D].rearrange("p (t d) -> p t d", d=D),
                        tmp[:, 0:S * D].rearrange(
                            "p (t w d) -> p t d w", w=w, d=D),
                        axis=AX.X)
                    aggn = spool.tile([P, CT_MAX * D], BF, tag="aggn", name="aggn")
                    nc.vector.tensor_tensor(
                        out=aggn[:, 0:ct * D].rearrange(
                            "p (t h e) -> p t h e", h=H, e=HD),
                        in0=agg[:, 0:ct * D].rearrange(
                            "p (t h e) -> p t h e", h=H, e=HD),
                        in1=rec[:, 0:ct * H].rearrange("p (t h) -> p t h", h=H)
                            .unsqueeze(3).to_broadcast([P, ct, H, HD]),
                        op=OP.mult)
                    # fused output projection, one node tile at a time
                    for tl in range(ct):
                        nt = nt0 + tl
                        aT_ps = pspool_o.tile([P, P], BF, tag="aT", name="aT")
                        nc.tensor.transpose(
                            aT_ps[:], aggn[:, tl * D:(tl + 1) * D], ident[:])
                        aT = projpool.tile([P, P], BF, tag="aT_sb", name="aT_sb")
                        nc.vector.tensor_copy(aT[:], aT_ps[:])
                        if l < L - 1:
                            oT_ps = pspool_o.tile([P, P], FP, tag="oT", name="oT")
                            nc.tensor.matmul(
                                oT_ps[:], lhsT=wo_sb[:, l * D:(l + 1) * D],
                                rhs=aT[:], start=True, stop=True)
                            nc.vector.tensor_tensor(
                                out=xT_sb[:, nt * P:(nt + 1) * P],
                                in0=oT_ps[:],
                                in1=boT_sb[:, l:l + 1].to_broadcast([P, P]),
                                op=OP.add)
                        else:
                            o_ps = pspool_o.tile([P, nclass], FP, tag="o", name="o")
                            nc.tensor.matmul(
                                o_ps[:], lhsT=aT[:],
                                rhs=wo_sb[:, l * D:l * D + nclass],
                                start=True, stop=True)
                            o_sb = projpool.tile(
                                [P, nclass], FP, tag="o_sb", name="o_sb")
                            nc.vector.tensor_tensor(
                                out=o_sb[:], in0=o_ps[:], in1=bol_sb[:],
                                op=OP.add)
                            nc.sync.dma_start(
                                out_ext[nt * P:(nt + 1) * P, :], o_sb[:])
                    ecol += (128 * S) // 16
                    scol += S
                # tail tiles (deg-0 + padding rows): out = bias only
                for t in range(lay.n_class_tiles, T):
                    if l < L - 1:
                        nc.vector.tensor_copy(
                            xT_sb[:, t * P:(t + 1) * P],
                            boT_sb[:, l:l + 1].to_broadcast([P, P]))
                    else:
                        o_sb = projpool.tile(
                            [P, nclass], FP, tag="o_sb", name="o_sb")
                        nc.vector.tensor_copy(o_sb[:], bol_sb[:])
                        nc.sync.dma_start(out_ext[t * P:(t + 1) * P, :], o_sb[:])
    nc.compile()
    return nc


# ----------------------------------------------------------------------------
# Entry point
# ----------------------------------------------------------------------------

_trace = [False]  # test.py can flip this to profile


def kernel(x, src, dst, qkv_w, qkv_b, out_w, out_b, out_w_last, out_b_last):
    x = np.asarray(x, dtype=np.float32)
    lay = build_layout(np.asarray(src), np.asarray(dst), x.shape[0])
    nclass = np.asarray(out_w_last).shape[1]
    in_maps = host_inputs(
        lay, x, np.asarray(qkv_w, dtype=np.float32),
        np.asarray(qkv_b, dtype=np.float32), np.asarray(out_w, dtype=np.float32),
        np.asarray(out_b, dtype=np.float32),
        np.asarray(out_w_last, dtype=np.float32),
        np.asarray(out_b_last, dtype=np.float32),
    )
    nc = build_nc(lay, nclass)
    res = run_bass_kernel_spmd(
        nc, in_maps, core_ids=list(range(NCORES)), trace=_trace[0]
    )
    kernel.last_results = res
    outs = [np.asarray(res.results[c]["out"], dtype=np.float32)
            for c in range(NCORES)]
    return host_output(lay, outs, nclass)


# revision 18
# speedup vs baseline: 1.6519x; 1.1665x over previous
"""Graph-Transformer message-passing kernel for 8 Trainium2 NeuronCores.

Strategy (v2, dst-shard + batched SWDGE gathers, bf16):
  - Nodes split into 8 contiguous dst ranges; core c owns all edges into its
    range, so softmax/aggregation are fully local to a core.
  - Host groups each core's nodes into degree classes (DP-optimized widths),
    pads each node's in-edge list to the class width W; pad slots carry a
    -1e30 mask so they vanish in the softmax.
  - Per layer: each core projects Q|K|V for its rows (PE, bf16), the q|v
    halves are AllGathered into a global [8R, 256] bf16 table, each core then
    compacts the ~31.7K unique source rows it needs into a private <32768-row
    table (two dma_gather passes, int16 idx limit), and edge-slot q|v rows are
    batch-gathered from it (one dma_gather per ~32-slot chunk instead of one
    indirect DMA per edge slot: ~1us fixed SWDGE cost amortized 32x).
  - Masked softmax (no max-subtraction: |score| < 4 for this model family) and
    weighted aggregation run on DVE in bf16; output projection on PE keeps x
    feature-major (xT) so the next layer's QKV needs no transpose.
  - 3 layers fused in one NEFF; host inverts the node permutation.
"""

import numpy as np
import ml_dtypes

import concourse.bass as bass
import concourse.bacc as bacc
import concourse.mybir as mybir
import concourse.tile as tile
from concourse.masks import make_identity
from concourse.bass_utils import run_bass_kernel_spmd

NCORES = 8
L = 3
H = 8
D = 128
HD = D // H
SCALE = 1.0 / float(np.sqrt(HD))
NEG = -1.0e30
P = 128
LO_ROWS = 32768          # int16 idx limit for the low gather region
SLOTS_PER_CHUNK = 32     # edge-slot columns per dma_gather / DVE chunk
COMP_CHUNK = 1024        # rows per compaction dma_gather (SWDGE ring: <=65 descs/ring)
GATHER_MAX_SLOTS = 8     # 128*8 = 1024 idxs per edge dma_gather

FP = mybir.dt.float32
BF = mybir.dt.bfloat16
I16 = mybir.dt.int16
AX = mybir.AxisListType
OP = mybir.AluOpType
BFNP = ml_dtypes.bfloat16


def _wrap16(idxs):
    """Lay out idx stream positions j -> [j%16, j//16], replicated across all
    eight 16-partition groups (the Q7 ucode reads group 16..31 on queue 0)."""
    n = len(idxs)
    cols = (n + 15) // 16
    t = np.zeros((16, cols), dtype=np.int16)
    t[np.arange(n) % 16, np.arange(n) // 16] = idxs
    return np.tile(t, (8, 1))


class Layout:
    pass


def _pick_classes(deg, chunk, n_nodes):
    """DP over degree boundaries minimizing total padded slots."""
    dmax = int(deg.max())
    counts = np.zeros((NCORES, dmax + 1), dtype=np.int64)
    for c in range(NCORES):
        d = deg[c * chunk:min(n_nodes, (c + 1) * chunk)]
        counts[c] = np.bincount(d, minlength=dmax + 1)
    cum = counts.cumsum(axis=1)  # cum[c, w] = nodes with deg <= w

    def cost(lo, w):  # class covers degrees (lo, w]
        n = cum[:, w] - cum[:, lo]
        cap = ((n + P - 1) // P * P).max()
        return int(cap) * w

    INF = float("inf")
    best = [0.0] + [INF] * dmax
    prev = [0] * (dmax + 1)
    for w in range(1, dmax + 1):
        for lo in range(w):
            v = best[lo] + cost(lo, w)
            if v < best[w]:
                best[w], prev[w] = v, lo
    bounds = []
    w = dmax
    while w > 0:
        bounds.append(w)
        w = prev[w]
    return sorted(bounds)


def build_layout(src, dst, n_nodes):
    src = np.asarray(src).astype(np.int64)
    dst = np.asarray(dst).astype(np.int64)
    N = n_nodes
    chunk = (N + NCORES - 1) // NCORES

    deg = np.bincount(dst, minlength=N)
    order = np.argsort(dst, kind="stable")
    src_sorted = src[order]
    starts = np.zeros(N + 1, dtype=np.int64)
    np.cumsum(deg, out=starts[1:])

    classes = _pick_classes(deg, chunk, N)

    # per-core per-class node lists + uniform caps
    node_lists = {}
    for c in range(NCORES):
        nlo, nhi = c * chunk, min(N, (c + 1) * chunk)
        d = deg[nlo:nhi]
        lo = 0
        for w in classes:
            node_lists[(c, w)] = np.nonzero((d > lo) & (d <= w))[0] + nlo
            lo = w
    caps = {}
    for w in classes:
        cap = max(len(node_lists[(c, w)]) for c in range(NCORES))
        caps[w] = ((cap + P - 1) // P) * P

    n_class_rows = sum(caps.values())
    deg0_max = max(
        int((deg[c * chunk:min(N, (c + 1) * chunk)] == 0).sum())
        for c in range(NCORES)
    )
    R = ((n_class_rows + deg0_max + P - 1) // P) * P
    T = R // P

    base = {}
    b = 0
    for w in classes:
        base[w] = b
        b += caps[w]

    perm = np.full((NCORES, R), -1, dtype=np.int64)
    row_of = np.full(N, -1, dtype=np.int64)
    for c in range(NCORES):
        nlo, nhi = c * chunk, min(N, (c + 1) * chunk)
        for w in classes:
            nl = node_lists[(c, w)]
            perm[c, base[w]:base[w] + len(nl)] = nl
            row_of[nl] = c * R + base[w] + np.arange(len(nl))
        deg0 = np.nonzero(deg[nlo:nhi] == 0)[0] + nlo
        perm[c, n_class_rows:n_class_rows + len(deg0)] = deg0
        row_of[deg0] = c * R + n_class_rows + np.arange(len(deg0))

    # split rows into halves A/B so (a) each AllGather half overlaps other
    # work and (b) each half-table stays below the int16 idx limit
    TA = T // 2
    HR = TA * P
    HRB = R - HR
    assert NCORES * HR <= 32767 and NCORES * HRB <= 32767

    # edge-slot chunks (uniform across cores): (w, nt0, t_in_class, ct)
    chunks = []
    nt = 0
    for w in classes:
        tiles = caps[w] // P
        ct0 = max(1, SLOTS_PER_CHUNK // w)
        t = 0
        while t < tiles:
            ct = min(ct0, tiles - t)
            chunks.append((w, nt, t, ct))
            nt += ct
            t += ct
    n_class_tiles = nt

    # per-core: unique src rows -> compact table; edge idx + mask tables.
    # Unique rows are split by which half-table (A/B) holds them; idx values
    # are half-local (core*half_rows + local).
    uniq_a, uniq_b = [], []
    for c in range(NCORES):
        sel = (dst >= c * chunk) & (dst < (c + 1) * chunk)
        rows = np.unique(row_of[src[sel]])
        local = rows % R
        core = rows // R
        selA = local < HR
        uniq_a.append((rows[selA], core[selA] * HR + local[selA]))
        uniq_b.append((rows[~selA], core[~selA] * HRB + local[~selA] - HR))
    NA = max((len(u[0]) + P - 1) // P * P for u in uniq_a)
    NB = max((len(u[0]) + P - 1) // P * P for u in uniq_b)
    NU = NA + NB
    assert NU <= 32767, f"compact table {NU} exceeds int16"

    comp_idx = np.zeros((NCORES, NU), dtype=np.int64)
    cpos = np.zeros((NCORES, NCORES * R), dtype=np.int32)
    for c in range(NCORES):
        (rows_a, half_a), (rows_b, half_b) = uniq_a[c], uniq_b[c]
        comp_idx[c, :len(half_a)] = half_a
        comp_idx[c, NA:NA + len(half_b)] = half_b
        cpos[c, rows_a] = np.arange(len(rows_a))
        cpos[c, rows_b] = NA + np.arange(len(rows_b))

    # comp gather chunk list: (region, n, tc_base, col_ofs); region 0=A, 1=B
    comp_chunks = []
    for region, start, total in ((0, 0, NA), (1, NA, NB)):
        done = 0
        while done < total:
            n = min(COMP_CHUNK, total - done)
            comp_chunks.append(
                (region, n, start + done, (start + done) // 16)
            )
            done += n

    # edge idx + mask, chunk-slot-major: j = s*128 + p, s = t_loc*w + ws
    S_tot = sum(w * ct for (w, _, _, ct) in chunks)
    eidx = np.zeros((NCORES, 128 * S_tot), dtype=np.int64)
    emask = np.full((NCORES, P, S_tot), NEG, dtype=np.float32)
    for c in range(NCORES):
        # per-class [cap, w] idx/valid matrices, vectorized over nodes
        mats = {}
        for w in classes:
            cap = caps[w]
            nodes = perm[c, base[w]:base[w] + cap]
            nd = np.maximum(nodes, 0)
            dg = np.where(nodes >= 0, deg[nd], 0)
            im = np.zeros((cap, w), dtype=np.int64)
            vm = np.zeros((cap, w), dtype=bool)
            for ws in range(w):
                ok = ws < dg
                sidx = starts[nd] + ws
                im[ok, ws] = cpos[c, row_of[src_sorted[np.minimum(
                    sidx, len(src_sorted) - 1)]]][ok]
                vm[:, ws] = ok
            vm[nodes < 0, 0] = True  # pad row: 1 live slot, no NaN
            mats[w] = (im, vm)
        j0 = 0
        s0 = 0
        for (w, nt0, t_in_class, ct) in chunks:
            im, vm = mats[w]
            blk = slice(t_in_class * P, (t_in_class + ct) * P)
            imc = im[blk].reshape(ct, P, w)
            vmc = vm[blk].reshape(ct, P, w)
            n = ct * w * P
            eidx[c, j0:j0 + n] = imc.transpose(0, 2, 1).ravel()
            emask[c][:, s0:s0 + ct * w] = np.where(
                vmc.transpose(1, 0, 2).reshape(P, ct * w), 0.0, NEG)
            j0 += 128 * ct * w
            s0 += ct * w
    lay = Layout()
    lay.N, lay.R, lay.T, lay.chunk = N, R, T, chunk
    lay.classes, lay.caps, lay.base = classes, caps, base
    lay.n_class_tiles = n_class_tiles
    lay.chunks, lay.comp_chunks = chunks, comp_chunks
    lay.NA, lay.NB, lay.NU, lay.S_tot = NA, NB, NU, S_tot
    lay.TA, lay.HR, lay.HRB = TA, HR, HRB
    lay.perm, lay.row_of = perm, row_of
    lay.eidx, lay.emask, lay.comp_idx = eidx, emask, comp_idx
    return lay


def host_inputs(lay, x, qkv_w, qkv_b, out_w, out_b, out_w_last, out_b_last):
    x = np.asarray(x, dtype=np.float32)
    nclass = out_w_last.shape[1]
    wc = np.zeros((L, D, 3 * D), dtype=np.float32)
    bc = np.zeros((L, P, 3 * D), dtype=np.float32)
    wo = np.zeros((L, D, D), dtype=np.float32)
    boT = np.zeros((D, L), dtype=np.float32)
    bo_last = np.tile(out_b_last[None, :], (P, 1)).astype(np.float32)
    for l in range(L):
        wq, wk, wv = qkv_w[l, 0], qkv_w[l, 1], qkv_w[l, 2]
        bq, bk, bv = qkv_b[l, 0], qkv_b[l, 1], qkv_b[l, 2]
        wc[l] = np.concatenate([wq, wk * SCALE, wv], axis=1)
        bc[l] = np.tile(np.concatenate([bq, bk * SCALE, bv])[None, :], (P, 1))
        if l < L - 1:
            wo[l] = out_w[l]
            boT[:, l] = out_b[l]
        else:
            wo[l, :, :nclass] = out_w_last

    in_maps = []
    for c in range(NCORES):
        xp = np.where((lay.perm[c] >= 0)[:, None],
                      x[np.maximum(lay.perm[c], 0)], 0.0)
        m = {
            "x0T": np.ascontiguousarray(xp.T).astype(BFNP),
            "wc": wc.astype(BFNP), "bc": bc,
            "wo": wo.astype(BFNP), "boT": boT, "bo_last": bo_last,
            "eidx": _wrap16(lay.eidx[c]),
            "emask": lay.emask[c],
            "cidx": _wrap16(lay.comp_idx[c]),
        }
        in_maps.append(m)
    return in_maps


def host_output(lay, outs, nclass):
    full = np.zeros((lay.N, nclass), dtype=np.float32)
    for c in range(NCORES):
        real = lay.perm[c] >= 0
        full[lay.perm[c][real]] = outs[c][real]
    return full


# ----------------------------------------------------------------------------
# Device program
# ----------------------------------------------------------------------------

_PHASES = ["ABCD"]  # debug: phase bisect knob


def build_nc(lay, nclass):
    R, T = lay.R, lay.T
    nc = bacc.Bacc(trn_type="TRN2", num_devices=NCORES, num_swdge_queues=4)

    x0T = nc.dram_tensor("x0T", [D, R], BF, kind="ExternalInput")
    wc = nc.dram_tensor("wc", [L, D, 3 * D], BF, kind="ExternalInput")
    bc = nc.dram_tensor("bc", [L, P, 3 * D], FP, kind="ExternalInput")
    wo = nc.dram_tensor("wo", [L, D, D], BF, kind="ExternalInput")
    boT = nc.dram_tensor("boT", [D, L], FP, kind="ExternalInput")
    bo_last = nc.dram_tensor("bo_last", [P, nclass], FP, kind="ExternalInput")
    eidx_d = nc.dram_tensor(
        "eidx", [128, (128 * lay.S_tot) // 16], I16, kind="ExternalInput")
    emask_d = nc.dram_tensor("emask", [P, lay.S_tot], FP, kind="ExternalInput")
    cidx_d = nc.dram_tensor(
        "cidx", [128, lay.NU // 16], I16, kind="ExternalInput")
    out_ext = nc.dram_tensor("out", [R, nclass], FP, kind="ExternalOutput")

    TA, HR, HRB = lay.TA, lay.HR, lay.HRB
    qv_sliceA = nc.dram_tensor("qv_sliceA", [HR, 2 * D], BF, kind="Internal")
    qv_sliceB = nc.dram_tensor("qv_sliceB", [HRB, 2 * D], BF, kind="Internal")
    qv_fullA = nc.dram_tensor(
        "qv_fullA", [NCORES * HR, 2 * D], BF, kind="Internal", addr_space="Shared"
    )
    qv_fullB = nc.dram_tensor(
        "qv_fullB", [NCORES * HRB, 2 * D], BF, kind="Internal", addr_space="Shared"
    )
    tcomp = nc.dram_tensor("tcomp", [lay.NU, 2 * D], BF, kind="Internal")
    rg = [list(range(NCORES))]

    CT_MAX = max(ct for (_, _, _, ct) in lay.chunks)
    with tile.TileContext(nc) as tc:
        with (
            tc.tile_pool(name="const", bufs=1) as cpool,
            tc.tile_pool(name="persist", bufs=1) as ppool,
            tc.tile_pool(name="proj", bufs=4) as projpool,
            tc.tile_pool(name="work", bufs=2) as wpool,
            tc.tile_pool(name="small", bufs=3) as spool,
            tc.tile_pool(name="psum", bufs=2, space="PSUM") as pspool,
            tc.tile_pool(name="psum_o", bufs=2, space="PSUM") as pspool_o,
        ):
            ident = cpool.tile([P, P], BF, tag="ident", name="ident")
            make_identity(nc, ident[:])
            wc_sb = cpool.tile([P, L * 3 * D], BF, tag="wc", name="wc")
            nc.sync.dma_start(
                wc_sb[:].rearrange("k (l n) -> k l n", l=L),
                wc[:].rearrange("l k n -> k l n"))
            bc_sb = cpool.tile([P, L * 3 * D], FP, tag="bc", name="bc")
            nc.sync.dma_start(
                bc_sb[:].rearrange("p (l n) -> p l n", l=L),
                bc[:].rearrange("l p n -> p l n"))
            wo_sb = cpool.tile([P, L * D], BF, tag="wo", name="wo")
            nc.sync.dma_start(
                wo_sb[:].rearrange("k (l n) -> k l n", l=L),
                wo[:].rearrange("l k n -> k l n"))
            boT_sb = cpool.tile([P, L], FP, tag="boT", name="boT")
            nc.sync.dma_start(boT_sb[:], boT[:])
            bol_sb = cpool.tile([P, nclass], FP, tag="bol", name="bol")
            nc.sync.dma_start(bol_sb[:], bo_last[:])
            eidx_sb = cpool.tile(
                [128, (128 * lay.S_tot) // 16], I16, tag="eidx", name="eidx")
            nc.sync.dma_start(eidx_sb[:], eidx_d[:])
            mask_sb = cpool.tile([P, lay.S_tot], FP, tag="mask", name="mask")
            nc.sync.dma_start(mask_sb[:], emask_d[:])
            cidx_sb = cpool.tile([128, lay.NU // 16], I16, tag="cidx", name="cidx")
            nc.sync.dma_start(cidx_sb[:], cidx_d[:])

            qrr = [0]  # SWDGE queue round-robin

            def next_q():
                qrr[0] = (qrr[0] + 1) % 4
                return qrr[0]

            xT_sb = ppool.tile([P, R], BF, tag="x", name="x")
            nc.sync.dma_start(xT_sb[:], x0T[:])
            k_sb = ppool.tile([P, T * D], BF, tag="k", name="k")
            tc.strict_bb_all_engine_barrier()

            for l in range(L):
                # ---- Phase A: QKV projection (x kept feature-major) ----
                def proj_tile(t):
                    qkv_ps = pspool.tile([P, 3 * D], FP, tag="qkv", name="qkv")
                    nc.tensor.matmul(
                        qkv_ps[:], lhsT=xT_sb[:, t * P:(t + 1) * P],
                        rhs=wc_sb[:, l * 3 * D:(l + 1) * 3 * D],
                        start=True, stop=True,
                    )
                    bofs = l * 3 * D
                    qv_st = projpool.tile([P, 2 * D], BF, tag="qv_st", name="qv_st")
                    nc.vector.tensor_tensor(
                        out=qv_st[:, 0:D], in0=qkv_ps[:, 0:D],
                        in1=bc_sb[:, bofs:bofs + D], op=OP.add)
                    nc.vector.tensor_tensor(
                        out=qv_st[:, D:2 * D], in0=qkv_ps[:, 2 * D:3 * D],
                        in1=bc_sb[:, bofs + 2 * D:bofs + 3 * D], op=OP.add)
                    nc.vector.tensor_tensor(
                        out=k_sb[:, t * D:(t + 1) * D], in0=qkv_ps[:, D:2 * D],
                        in1=bc_sb[:, bofs + D:bofs + 2 * D], op=OP.add)
                    if t < TA:
                        nc.sync.dma_start(qv_sliceA[t * P:(t + 1) * P, :], qv_st[:])
                    else:
                        nc.sync.dma_start(
                            qv_sliceB[(t - TA) * P:(t - TA + 1) * P, :], qv_st[:])

                def comp_region(region):
                    # compact the unique src rows of one half-table
                    for (rgn, n, tc_base, col_ofs) in (
                            lay.comp_chunks if "C" in _PHASES[0] else []):
                        if rgn != region:
                            continue
                        cg = wpool.tile(
                            [P, COMP_CHUNK // P * 2 * D], BF, tag="cg", name="cg")
                        sl = n // P * 2 * D
                        nc.gpsimd.dma_gather(
                            cg[:, 0:sl].rearrange("p (s d) -> p s d", d=2 * D),
                            (qv_fullA if region == 0 else qv_fullB)[:, :],
                            cidx_sb[:, col_ofs:col_ofs + n // 16],
                            n, n, 2 * D, queue_num=next_q(),
                        )
                        nc.scalar.dma_start(
                            tcomp[tc_base:tc_base + n, :].rearrange(
                                "(s p) d -> p s d", p=P),
                            cg[:, 0:sl].rearrange("p (s d) -> p s d", d=2 * D),
                        )

                for t in range(TA):
                    proj_tile(t)
                # ---- Phase B1: exchange first half (overlaps proj B) ----
                nc.gpsimd.collective_compute(
                    "AllGather", OP.bypass, replica_groups=rg,
                    ins=[qv_sliceA[:, :]], outs=[qv_fullA[:, :]],
                )
                comp_region(0)  # overlaps proj B / CC2 on the Pool queue
                for t in range(TA, T):
                    proj_tile(t)
                # ---- Phase B2: exchange second half ----
                nc.gpsimd.collective_compute(
                    "AllGather", OP.bypass, replica_groups=rg,
                    ins=[qv_sliceB[:, :]], outs=[qv_fullB[:, :]],
                )
                comp_region(1)

                # ---- Phase D: edge softmax + aggregation + out projection ----
                ecol = 0
                scol = 0
                for (w, nt0, t_in_class, ct) in (
                        lay.chunks if "D" in _PHASES[0] else []):
                    S = ct * w
                    qv_g = wpool.tile(
                        [P, SLOTS_PER_CHUNK * 2 * D], BF, tag="qvg", name="qvg")
                    off = 0
                    while off < S:
                        gs = min(GATHER_MAX_SLOTS, S - off)
                        nc.gpsimd.dma_gather(
                            qv_g[:, off * 2 * D:(off + gs) * 2 * D]
                                .rearrange("p (s d) -> p s d", d=2 * D),
                            tcomp[:, :],
                            eidx_sb[:, ecol + off * 8:ecol + (off + gs) * 8],
                            128 * gs, 128 * gs, 2 * D, queue_num=next_q(),
                        )
                        off += gs
                    # scores: per-head dot(q_gathered, k_local)
                    tmp = wpool.tile(
                        [P, SLOTS_PER_CHUNK * D], BF, tag="tmp", name="tmp")
                    q4 = qv_g[:, 0:S * 2 * D].rearrange(
                        "p (t w c) -> p t w c", w=w, c=2 * D)[:, :, :, 0:D]
                    k4 = (k_sb[:, nt0 * D:(nt0 + ct) * D]
                          .rearrange("p (t d) -> p t d", d=D)
                          .unsqueeze(2).to_broadcast([P, ct, w, D]))
                    nc.vector.tensor_tensor(
                        out=tmp[:, 0:S * D].rearrange(
                            "p (t w d) -> p t w d", w=w, d=D),
                        in0=q4, in1=k4, op=OP.mult)
                    s_t = spool.tile([P, SLOTS_PER_CHUNK * H], FP, tag="s", name="s")
                    nc.vector.reduce_sum(
                        s_t[:, 0:S * H].rearrange("p (s h) -> p s h", h=H),
                        tmp[:, 0:S * D].rearrange(
                            "p (s h e) -> p s h e", h=H, e=HD),
                        axis=AX.X)
                    nc.vector.tensor_tensor(
                        out=s_t[:, 0:S * H].rearrange("p (s h) -> p s h", h=H),
                        in0=s_t[:, 0:S * H].rearrange("p (s h) -> p s h", h=H),
                        in1=mask_sb[:, scol:scol + S]
                            .unsqueeze(2).to_broadcast([P, S, H]),
                        op=OP.add)
                    ex = spool.tile([P, SLOTS_PER_CHUNK * H], BF, tag="ex", name="ex")
                    nc.scalar.activation(
                        out=ex[:, 0:S * H], in_=s_t[:, 0:S * H],
                        func=mybir.ActivationFunctionType.Exp)
                    denom = spool.tile([P, CT_MAX * H], FP, tag="dn", name="dn")
                    nc.vector.reduce_sum(
                        denom[:, 0:ct * H].rearrange("p (t h) -> p t h", h=H),
                        ex[:, 0:S * H].rearrange(
                            "p (t w h) -> p t h w", w=w, h=H),
                        axis=AX.X)
                    rec = spool.tile([P, CT_MAX * H], FP, tag="rc", name="rc")
                    nc.vector.reciprocal(rec[:, 0:ct * H], denom[:, 0:ct * H])
                    # weighted v
                    nc.vector.tensor_tensor(
                        out=tmp[:, 0:S * D].rearrange(
                            "p (s h e) -> p s h e", h=H, e=HD),
                        in0=qv_g[:, 0:S * 2 * D].rearrange(
                            "p (s c) -> p s c", c=2 * D)[:, :, D:2 * D]
                            .rearrange("p s (h e) -> p s h e", h=H),
                        in1=ex[:, 0:S * H].rearrange("p (s h) -> p s h", h=H)
                            .unsqueeze(3).to_broadcast([P, S, H, HD]),
                        op=OP.mult)
                    # reduce over w via pairwise tree adds (contiguous runs
                    # beat the strided reduce_sum by ~4x on DVE)
                    agg = spool.tile([P, CT_MAX * D], FP, tag="agg", name="agg")
                    tview = tmp[:, 0:S * D].rearrange(
                        "p (t w d) -> p t w d", w=w, d=D)
                    n = w
                    while n > 2:
                        h = n // 2
                        nc.vector.tensor_tensor(
                            out=tview[:, :, 0:h, :], in0=tview[:, :, 0:h, :],
                            in1=tview[:, :, n - h:n, :], op=OP.add)
                        n -= h
                    if n == 2:
                        nc.vector.tensor_tensor(
                            out=agg[:, 0:ct * D].rearrange(
                                "p (t z d) -> p t z d", t=ct, z=1),
                            in0=tview[:, :, 0:1, :], in1=tview[:, :, 1:2, :],
                            op=OP.add)
                    else:
                        nc.vector.tensor_copy(
                            agg[:, 0:ct * D].rearrange(
                                "p (t z d) -> p t z d", t=ct, z=1),
                            tview[:, :, 0:1, :])
                    aggn = spool.tile([P, CT_MAX * D], BF, tag="aggn", name="aggn")
                    nc.vector.tensor_tensor(
                        out=aggn[:, 0:ct * D].rearrange(
                            "p (t h e) -> p t h e", h=H, e=HD),
                        in0=agg[:, 0:ct * D].rearrange(
                            "p (t h e) -> p t h e", h=H, e=HD),
                        in1=rec[:, 0:ct * H].rearrange("p (t h) -> p t h", h=H)
                            .unsqueeze(3).to_broadcast([P, ct, H, HD]),
                        op=OP.mult)
                    # fused output projection, one node tile at a time
                    for tl in range(ct):
                        nt = nt0 + tl
                        aT_ps = pspool_o.tile([P, P], BF, tag="aT", name="aT")
                        nc.tensor.transpose(
                            aT_ps[:], aggn[:, tl * D:(tl + 1) * D], ident[:])
                        aT = projpool.tile([P, P], BF, tag="aT_sb", name="aT_sb")
                        nc.vector.tensor_copy(aT[:], aT_ps[:])
                        if l < L - 1:
                            oT_ps = pspool_o.tile([P, P], FP, tag="oT", name="oT")
                            nc.tensor.matmul(
                                oT_ps[:], lhsT=wo_sb[:, l * D:(l + 1) * D],
                                rhs=aT[:], start=True, stop=True)
                            nc.vector.tensor_tensor(
                                out=xT_sb[:, nt * P:(nt + 1) * P],
                                in0=oT_ps[:],
                                in1=boT_sb[:, l:l + 1].to_broadcast([P, P]),
                                op=OP.add)
                        else:
                            o_ps = pspool_o.tile([P, nclass], FP, tag="o", name="o")
                            nc.tensor.matmul(
                                o_ps[:], lhsT=aT[:],
                                rhs=wo_sb[:, l * D:l * D + nclass],
                                start=True, stop=True)
                            o_sb = projpool.tile(
                                [P, nclass], FP, tag="o_sb", name="o_sb")
                            nc.vector.tensor_tensor(
                                out=o_sb[:], in0=o_ps[:], in1=bol_sb[:],
                                op=OP.add)
                            nc.sync.dma_start(
                                out_ext[nt * P:(nt + 1) * P, :], o_sb[:])
                    ecol += (128 * S) // 16
                    scol += S
                # tail tiles (deg-0 + padding rows): out = bias only
                for t in range(lay.n_class_tiles, T):
                    if l < L - 1:
                        nc.vector.tensor_copy(
                            xT_sb[:, t * P:(t + 1) * P],
                            boT_sb[:, l:l + 1].to_broadcast([P, P]))
                    else:
                        o_sb = projpool.tile(
                            [P, nclass], FP, tag="o_sb", name="o_sb")
                        nc.vector.tensor_copy(o_sb[:], bol_sb[:])
                        nc.sync.dma_start(out_ext[t * P:(t + 1) * P, :], o_sb[:])
    nc.compile()
    return nc


# ----------------------------------------------------------------------------
# Entry point
# ----------------------------------------------------------------------------

_trace = [False]  # test.py can flip this to profile


def kernel(x, src, dst, qkv_w, qkv_b, out_w, out_b, out_w_last, out_b_last):
    x = np.asarray(x, dtype=np.float32)
    lay = build_layout(np.asarray(src), np.asarray(dst), x.shape[0])
    nclass = np.asarray(out_w_last).shape[1]
    in_maps = host_inputs(
        lay, x, np.asarray(qkv_w, dtype=np.float32),
        np.asarray(qkv_b, dtype=np.float32), np.asarray(out_w, dtype=np.float32),
        np.asarray(out_b, dtype=np.float32),
        np.asarray(out_w_last, dtype=np.float32),
        np.asarray(out_b_last, dtype=np.float32),
    )
    nc = build_nc(lay, nclass)
    res = run_bass_kernel_spmd(
        nc, in_maps, core_ids=list(range(NCORES)), trace=_trace[0]
    )
    kernel.last_results = res
    outs = [np.asarray(res.results[c]["out"], dtype=np.float32)
            for c in range(NCORES)]
    return host_output(lay, outs, nclass)


# revision 19
# speedup vs baseline: 1.6810x; 1.0176x over previous
"""Graph-Transformer message-passing kernel for 8 Trainium2 NeuronCores.

Strategy (v2, dst-shard + batched SWDGE gathers, bf16):
  - Nodes split into 8 contiguous dst ranges; core c owns all edges into its
    range, so softmax/aggregation are fully local to a core.
  - Host groups each core's nodes into degree classes (DP-optimized widths),
    pads each node's in-edge list to the class width W; pad slots carry a
    -1e30 mask so they vanish in the softmax.
  - Per layer: each core projects Q|K|V for its rows (PE, bf16), the q|v
    halves are AllGathered into a global [8R, 256] bf16 table, each core then
    compacts the ~31.7K unique source rows it needs into a private <32768-row
    table (two dma_gather passes, int16 idx limit), and edge-slot q|v rows are
    batch-gathered from it (one dma_gather per ~32-slot chunk instead of one
    indirect DMA per edge slot: ~1us fixed SWDGE cost amortized 32x).
  - Masked softmax (no max-subtraction: |score| < 4 for this model family) and
    weighted aggregation run on DVE in bf16; output projection on PE keeps x
    feature-major (xT) so the next layer's QKV needs no transpose.
  - 3 layers fused in one NEFF; host inverts the node permutation.
"""

import numpy as np
import ml_dtypes

import concourse.bass as bass
import concourse.bacc as bacc
import concourse.mybir as mybir
import concourse.tile as tile
from concourse.masks import make_identity
from concourse.bass_utils import run_bass_kernel_spmd

NCORES = 8
L = 3
H = 8
D = 128
HD = D // H
SCALE = 1.0 / float(np.sqrt(HD))
NEG = -1.0e30
P = 128
LO_ROWS = 32768          # int16 idx limit for the low gather region
SLOTS_PER_CHUNK = 48     # edge-slot columns per DVE chunk
COMP_CHUNK = 1024        # rows per compaction dma_gather (SWDGE ring: <=65 descs/ring)
GATHER_MAX_SLOTS = 8     # 128*8 = 1024 idxs per edge dma_gather

FP = mybir.dt.float32
BF = mybir.dt.bfloat16
I16 = mybir.dt.int16
AX = mybir.AxisListType
OP = mybir.AluOpType
BFNP = ml_dtypes.bfloat16


def _wrap16(idxs):
    """Lay out idx stream positions j -> [j%16, j//16], replicated across all
    eight 16-partition groups (the Q7 ucode reads group 16..31 on queue 0)."""
    n = len(idxs)
    cols = (n + 15) // 16
    t = np.zeros((16, cols), dtype=np.int16)
    t[np.arange(n) % 16, np.arange(n) // 16] = idxs
    return np.tile(t, (8, 1))


class Layout:
    pass


def _pick_classes(deg, chunk, n_nodes):
    """DP over degree boundaries minimizing total padded slots."""
    dmax = int(deg.max())
    counts = np.zeros((NCORES, dmax + 1), dtype=np.int64)
    for c in range(NCORES):
        d = deg[c * chunk:min(n_nodes, (c + 1) * chunk)]
        counts[c] = np.bincount(d, minlength=dmax + 1)
    cum = counts.cumsum(axis=1)  # cum[c, w] = nodes with deg <= w

    def cost(lo, w):  # class covers degrees (lo, w]
        n = cum[:, w] - cum[:, lo]
        cap = ((n + P - 1) // P * P).max()
        return int(cap) * w

    INF = float("inf")
    best = [0.0] + [INF] * dmax
    prev = [0] * (dmax + 1)
    for w in range(1, dmax + 1):
        for lo in range(w):
            v = best[lo] + cost(lo, w)
            if v < best[w]:
                best[w], prev[w] = v, lo
    bounds = []
    w = dmax
    while w > 0:
        bounds.append(w)
        w = prev[w]
    return sorted(bounds)


def build_layout(src, dst, n_nodes):
    src = np.asarray(src).astype(np.int64)
    dst = np.asarray(dst).astype(np.int64)
    N = n_nodes
    chunk = (N + NCORES - 1) // NCORES

    deg = np.bincount(dst, minlength=N)
    order = np.argsort(dst, kind="stable")
    src_sorted = src[order]
    starts = np.zeros(N + 1, dtype=np.int64)
    np.cumsum(deg, out=starts[1:])

    classes = _pick_classes(deg, chunk, N)

    # per-core per-class node lists + uniform caps
    node_lists = {}
    for c in range(NCORES):
        nlo, nhi = c * chunk, min(N, (c + 1) * chunk)
        d = deg[nlo:nhi]
        lo = 0
        for w in classes:
            node_lists[(c, w)] = np.nonzero((d > lo) & (d <= w))[0] + nlo
            lo = w
    caps = {}
    for w in classes:
        cap = max(len(node_lists[(c, w)]) for c in range(NCORES))
        caps[w] = ((cap + P - 1) // P) * P

    n_class_rows = sum(caps.values())
    deg0_max = max(
        int((deg[c * chunk:min(N, (c + 1) * chunk)] == 0).sum())
        for c in range(NCORES)
    )
    R = ((n_class_rows + deg0_max + P - 1) // P) * P
    T = R // P

    base = {}
    b = 0
    for w in classes:
        base[w] = b
        b += caps[w]

    perm = np.full((NCORES, R), -1, dtype=np.int64)
    row_of = np.full(N, -1, dtype=np.int64)
    for c in range(NCORES):
        nlo, nhi = c * chunk, min(N, (c + 1) * chunk)
        for w in classes:
            nl = node_lists[(c, w)]
            perm[c, base[w]:base[w] + len(nl)] = nl
            row_of[nl] = c * R + base[w] + np.arange(len(nl))
        deg0 = np.nonzero(deg[nlo:nhi] == 0)[0] + nlo
        perm[c, n_class_rows:n_class_rows + len(deg0)] = deg0
        row_of[deg0] = c * R + n_class_rows + np.arange(len(deg0))

    # split rows into NREG regions so (a) each AllGather part overlaps other
    # work and (b) each part-table stays below the int16 idx limit
    NREG = 4
    tile_cnt = [T // NREG + (1 if i < T % NREG else 0) for i in range(NREG)]
    treg0 = np.cumsum([0] + tile_cnt)  # tile offsets per region
    hrows = [tc_ * P for tc_ in tile_cnt]
    hbase = [int(treg0[i]) * P for i in range(NREG)]
    assert all(NCORES * h <= 32767 for h in hrows)

    # edge-slot chunks (uniform across cores): (w, nt0, t_in_class, ct)
    chunks = []
    nt = 0
    for w in classes:
        tiles = caps[w] // P
        ct0 = max(1, SLOTS_PER_CHUNK // w)
        t = 0
        while t < tiles:
            ct = min(ct0, tiles - t)
            chunks.append((w, nt, t, ct))
            nt += ct
            t += ct
    n_class_tiles = nt

    # per-core: unique src rows -> compact table; edge idx + mask tables.
    # Unique rows are split by which half-table (A/B) holds them; idx values
    # are half-local (core*half_rows + local).
    uniq = [[] for _ in range(NREG)]  # per region: list over cores of (rows, halfidx)
    for c in range(NCORES):
        sel = (dst >= c * chunk) & (dst < (c + 1) * chunk)
        rows = np.unique(row_of[src[sel]])
        local = rows % R
        core = rows // R
        for i in range(NREG):
            inr = (local >= hbase[i]) & (local < hbase[i] + hrows[i])
            uniq[i].append(
                (rows[inr], core[inr] * hrows[i] + local[inr] - hbase[i]))
    NR = [max((len(u[0]) + P - 1) // P * P for u in uniq[i])
          for i in range(NREG)]
    NU = sum(NR)
    assert NU <= 32767, f"compact table {NU} exceeds int16"
    nrbase = np.cumsum([0] + NR)

    comp_idx = np.zeros((NCORES, NU), dtype=np.int64)
    cpos = np.zeros((NCORES, NCORES * R), dtype=np.int32)
    for c in range(NCORES):
        for i in range(NREG):
            rows_i, half_i = uniq[i][c]
            comp_idx[c, nrbase[i]:nrbase[i] + len(half_i)] = half_i
            cpos[c, rows_i] = nrbase[i] + np.arange(len(rows_i))

    # comp gather chunk list: (region, n, tc_base, col_ofs)
    comp_chunks = []
    for i in range(NREG):
        done = 0
        while done < NR[i]:
            n = min(COMP_CHUNK, NR[i] - done)
            b = int(nrbase[i]) + done
            comp_chunks.append((i, n, b, b // 16))
            done += n

    # edge idx + mask, chunk-slot-major: j = s*128 + p, s = t_loc*w + ws
    S_tot = sum(w * ct for (w, _, _, ct) in chunks)
    eidx = np.zeros((NCORES, 128 * S_tot), dtype=np.int64)
    emask = np.full((NCORES, P, S_tot), NEG, dtype=np.float32)
    for c in range(NCORES):
        # per-class [cap, w] idx/valid matrices, vectorized over nodes
        mats = {}
        for w in classes:
            cap = caps[w]
            nodes = perm[c, base[w]:base[w] + cap]
            nd = np.maximum(nodes, 0)
            dg = np.where(nodes >= 0, deg[nd], 0)
            im = np.zeros((cap, w), dtype=np.int64)
            vm = np.zeros((cap, w), dtype=bool)
            for ws in range(w):
                ok = ws < dg
                sidx = starts[nd] + ws
                im[ok, ws] = cpos[c, row_of[src_sorted[np.minimum(
                    sidx, len(src_sorted) - 1)]]][ok]
                vm[:, ws] = ok
            vm[nodes < 0, 0] = True  # pad row: 1 live slot, no NaN
            mats[w] = (im, vm)
        j0 = 0
        s0 = 0
        for (w, nt0, t_in_class, ct) in chunks:
            im, vm = mats[w]
            blk = slice(t_in_class * P, (t_in_class + ct) * P)
            imc = im[blk].reshape(ct, P, w)
            vmc = vm[blk].reshape(ct, P, w)
            n = ct * w * P
            eidx[c, j0:j0 + n] = imc.transpose(0, 2, 1).ravel()
            emask[c][:, s0:s0 + ct * w] = np.where(
                vmc.transpose(1, 0, 2).reshape(P, ct * w), 0.0, NEG)
            j0 += 128 * ct * w
            s0 += ct * w
    lay = Layout()
    lay.N, lay.R, lay.T, lay.chunk = N, R, T, chunk
    lay.classes, lay.caps, lay.base = classes, caps, base
    lay.n_class_tiles = n_class_tiles
    lay.chunks, lay.comp_chunks = chunks, comp_chunks
    lay.NU, lay.S_tot = NU, S_tot
    lay.NREG, lay.tile_cnt, lay.treg0, lay.hrows = NREG, tile_cnt, treg0, hrows
    lay.perm, lay.row_of = perm, row_of
    lay.eidx, lay.emask, lay.comp_idx = eidx, emask, comp_idx
    return lay


def host_inputs(lay, x, qkv_w, qkv_b, out_w, out_b, out_w_last, out_b_last):
    x = np.asarray(x, dtype=np.float32)
    nclass = out_w_last.shape[1]
    wc = np.zeros((L, D, 3 * D), dtype=np.float32)
    bc = np.zeros((L, P, 3 * D), dtype=np.float32)
    wo = np.zeros((L, D, D), dtype=np.float32)
    boT = np.zeros((D, L), dtype=np.float32)
    bo_last = np.tile(out_b_last[None, :], (P, 1)).astype(np.float32)
    for l in range(L):
        wq, wk, wv = qkv_w[l, 0], qkv_w[l, 1], qkv_w[l, 2]
        bq, bk, bv = qkv_b[l, 0], qkv_b[l, 1], qkv_b[l, 2]
        wc[l] = np.concatenate([wq, wk * SCALE, wv], axis=1)
        bc[l] = np.tile(np.concatenate([bq, bk * SCALE, bv])[None, :], (P, 1))
        if l < L - 1:
            wo[l] = out_w[l]
            boT[:, l] = out_b[l]
        else:
            wo[l, :, :nclass] = out_w_last

    in_maps = []
    for c in range(NCORES):
        xp = np.where((lay.perm[c] >= 0)[:, None],
                      x[np.maximum(lay.perm[c], 0)], 0.0)
        m = {
            "x0T": np.ascontiguousarray(xp.T).astype(BFNP),
            "wc": wc.astype(BFNP), "bc": bc,
            "wo": wo.astype(BFNP), "boT": boT, "bo_last": bo_last,
            "eidx": _wrap16(lay.eidx[c]),
            "emask": lay.emask[c],
            "cidx": _wrap16(lay.comp_idx[c]),
        }
        in_maps.append(m)
    return in_maps


def host_output(lay, outs, nclass):
    full = np.zeros((lay.N, nclass), dtype=np.float32)
    for c in range(NCORES):
        real = lay.perm[c] >= 0
        full[lay.perm[c][real]] = outs[c][real]
    return full


# ----------------------------------------------------------------------------
# Device program
# ----------------------------------------------------------------------------

_PHASES = ["ABCD"]  # debug: phase bisect knob


def build_nc(lay, nclass):
    R, T = lay.R, lay.T
    nc = bacc.Bacc(trn_type="TRN2", num_devices=NCORES, num_swdge_queues=4)

    x0T = nc.dram_tensor("x0T", [D, R], BF, kind="ExternalInput")
    wc = nc.dram_tensor("wc", [L, D, 3 * D], BF, kind="ExternalInput")
    bc = nc.dram_tensor("bc", [L, P, 3 * D], FP, kind="ExternalInput")
    wo = nc.dram_tensor("wo", [L, D, D], BF, kind="ExternalInput")
    boT = nc.dram_tensor("boT", [D, L], FP, kind="ExternalInput")
    bo_last = nc.dram_tensor("bo_last", [P, nclass], FP, kind="ExternalInput")
    eidx_d = nc.dram_tensor(
        "eidx", [128, (128 * lay.S_tot) // 16], I16, kind="ExternalInput")
    emask_d = nc.dram_tensor("emask", [P, lay.S_tot], FP, kind="ExternalInput")
    cidx_d = nc.dram_tensor(
        "cidx", [128, lay.NU // 16], I16, kind="ExternalInput")
    out_ext = nc.dram_tensor("out", [R, nclass], FP, kind="ExternalOutput")

    NREG, tile_cnt, treg0, hrows = lay.NREG, lay.tile_cnt, lay.treg0, lay.hrows
    qv_slices = [
        nc.dram_tensor(f"qv_slice{i}", [hrows[i], 2 * D], BF, kind="Internal")
        for i in range(NREG)]
    qv_fulls = [
        nc.dram_tensor(f"qv_full{i}", [NCORES * hrows[i], 2 * D], BF,
                       kind="Internal", addr_space="Shared")
        for i in range(NREG)]
    tcomp = nc.dram_tensor("tcomp", [lay.NU, 2 * D], BF, kind="Internal")
    rg = [list(range(NCORES))]

    CT_MAX = max(ct for (_, _, _, ct) in lay.chunks)
    with tile.TileContext(nc) as tc:
        with (
            tc.tile_pool(name="const", bufs=1) as cpool,
            tc.tile_pool(name="persist", bufs=1) as ppool,
            tc.tile_pool(name="proj", bufs=4) as projpool,
            tc.tile_pool(name="work", bufs=2) as wpool,
            tc.tile_pool(name="small", bufs=3) as spool,
            tc.tile_pool(name="psum", bufs=2, space="PSUM") as pspool,
            tc.tile_pool(name="psum_o", bufs=2, space="PSUM") as pspool_o,
        ):
            ident = cpool.tile([P, P], BF, tag="ident", name="ident")
            make_identity(nc, ident[:])
            wc_sb = cpool.tile([P, L * 3 * D], BF, tag="wc", name="wc")
            nc.sync.dma_start(
                wc_sb[:].rearrange("k (l n) -> k l n", l=L),
                wc[:].rearrange("l k n -> k l n"))
            bc_sb = cpool.tile([P, L * 3 * D], FP, tag="bc", name="bc")
            nc.sync.dma_start(
                bc_sb[:].rearrange("p (l n) -> p l n", l=L),
                bc[:].rearrange("l p n -> p l n"))
            wo_sb = cpool.tile([P, L * D], BF, tag="wo", name="wo")
            nc.sync.dma_start(
                wo_sb[:].rearrange("k (l n) -> k l n", l=L),
                wo[:].rearrange("l k n -> k l n"))
            boT_sb = cpool.tile([P, L], FP, tag="boT", name="boT")
            nc.sync.dma_start(boT_sb[:], boT[:])
            bol_sb = cpool.tile([P, nclass], FP, tag="bol", name="bol")
            nc.sync.dma_start(bol_sb[:], bo_last[:])
            eidx_sb = cpool.tile(
                [128, (128 * lay.S_tot) // 16], I16, tag="eidx", name="eidx")
            nc.sync.dma_start(eidx_sb[:], eidx_d[:])
            mask_sb = cpool.tile([P, lay.S_tot], FP, tag="mask", name="mask")
            nc.sync.dma_start(mask_sb[:], emask_d[:])
            cidx_sb = cpool.tile([128, lay.NU // 16], I16, tag="cidx", name="cidx")
            nc.sync.dma_start(cidx_sb[:], cidx_d[:])

            qrr = [0]  # SWDGE queue round-robin

            def next_q():
                qrr[0] = (qrr[0] + 1) % 4
                return qrr[0]

            xT_sb = ppool.tile([P, R], BF, tag="x", name="x")
            nc.sync.dma_start(xT_sb[:], x0T[:])
            k_sb = ppool.tile([P, T * D], BF, tag="k", name="k")
            tc.strict_bb_all_engine_barrier()

            for l in range(L):
                # ---- Phase A: QKV projection (x kept feature-major) ----
                def proj_tile(t):
                    qkv_ps = pspool.tile([P, 3 * D], FP, tag="qkv", name="qkv")
                    nc.tensor.matmul(
                        qkv_ps[:], lhsT=xT_sb[:, t * P:(t + 1) * P],
                        rhs=wc_sb[:, l * 3 * D:(l + 1) * 3 * D],
                        start=True, stop=True,
                    )
                    bofs = l * 3 * D
                    qv_st = projpool.tile([P, 2 * D], BF, tag="qv_st", name="qv_st")
                    nc.vector.tensor_tensor(
                        out=qv_st[:, 0:D], in0=qkv_ps[:, 0:D],
                        in1=bc_sb[:, bofs:bofs + D], op=OP.add)
                    nc.vector.tensor_tensor(
                        out=qv_st[:, D:2 * D], in0=qkv_ps[:, 2 * D:3 * D],
                        in1=bc_sb[:, bofs + 2 * D:bofs + 3 * D], op=OP.add)
                    nc.vector.tensor_tensor(
                        out=k_sb[:, t * D:(t + 1) * D], in0=qkv_ps[:, D:2 * D],
                        in1=bc_sb[:, bofs + D:bofs + 2 * D], op=OP.add)
                    ri = int(np.searchsorted(treg0, t, side="right")) - 1
                    tl0 = t - int(treg0[ri])
                    nc.sync.dma_start(
                        qv_slices[ri][tl0 * P:(tl0 + 1) * P, :], qv_st[:])

                def comp_region(region):
                    # compact the unique src rows of one part-table
                    for (rgn, n, tc_base, col_ofs) in (
                            lay.comp_chunks if "C" in _PHASES[0] else []):
                        if rgn != region:
                            continue
                        cg = wpool.tile(
                            [P, COMP_CHUNK // P * 2 * D], BF, tag="cg", name="cg")
                        sl = n // P * 2 * D
                        nc.gpsimd.dma_gather(
                            cg[:, 0:sl].rearrange("p (s d) -> p s d", d=2 * D),
                            qv_fulls[region][:, :],
                            cidx_sb[:, col_ofs:col_ofs + n // 16],
                            n, n, 2 * D, queue_num=next_q(),
                        )
                        nc.scalar.dma_start(
                            tcomp[tc_base:tc_base + n, :].rearrange(
                                "(s p) d -> p s d", p=P),
                            cg[:, 0:sl].rearrange("p (s d) -> p s d", d=2 * D),
                        )

                # projection regions interleaved with AllGathers + compaction:
                # CC_i is issued right after region i's tiles project; its
                # compaction gathers then overlap later regions' CC transfers
                for i in range(NREG):
                    for t in range(int(treg0[i]), int(treg0[i + 1])):
                        proj_tile(t)
                    nc.gpsimd.collective_compute(
                        "AllGather", OP.bypass, replica_groups=rg,
                        ins=[qv_slices[i][:, :]], outs=[qv_fulls[i][:, :]],
                    )
                    comp_region(i)

                # ---- Phase D: edge softmax + aggregation + out projection ----
                ecol = 0
                scol = 0
                for (w, nt0, t_in_class, ct) in (
                        lay.chunks if "D" in _PHASES[0] else []):
                    S = ct * w
                    qv_g = wpool.tile(
                        [P, SLOTS_PER_CHUNK * 2 * D], BF, tag="qvg", name="qvg")
                    off = 0
                    while off < S:
                        gs = min(GATHER_MAX_SLOTS, S - off)
                        nc.gpsimd.dma_gather(
                            qv_g[:, off * 2 * D:(off + gs) * 2 * D]
                                .rearrange("p (s d) -> p s d", d=2 * D),
                            tcomp[:, :],
                            eidx_sb[:, ecol + off * 8:ecol + (off + gs) * 8],
                            128 * gs, 128 * gs, 2 * D, queue_num=next_q(),
                        )
                        off += gs
                    # scores: per-head dot(q_gathered, k_local)
                    tmp = wpool.tile(
                        [P, SLOTS_PER_CHUNK * D], BF, tag="tmp", name="tmp")
                    q4 = qv_g[:, 0:S * 2 * D].rearrange(
                        "p (t w c) -> p t w c", w=w, c=2 * D)[:, :, :, 0:D]
                    k4 = (k_sb[:, nt0 * D:(nt0 + ct) * D]
                          .rearrange("p (t d) -> p t d", d=D)
                          .unsqueeze(2).to_broadcast([P, ct, w, D]))
                    nc.vector.tensor_tensor(
                        out=tmp[:, 0:S * D].rearrange(
                            "p (t w d) -> p t w d", w=w, d=D),
                        in0=q4, in1=k4, op=OP.mult)
                    s_t = spool.tile([P, SLOTS_PER_CHUNK * H], FP, tag="s", name="s")
                    nc.vector.reduce_sum(
                        s_t[:, 0:S * H].rearrange("p (s h) -> p s h", h=H),
                        tmp[:, 0:S * D].rearrange(
                            "p (s h e) -> p s h e", h=H, e=HD),
                        axis=AX.X)
                    nc.vector.tensor_tensor(
                        out=s_t[:, 0:S * H].rearrange("p (s h) -> p s h", h=H),
                        in0=s_t[:, 0:S * H].rearrange("p (s h) -> p s h", h=H),
                        in1=mask_sb[:, scol:scol + S]
                            .unsqueeze(2).to_broadcast([P, S, H]),
                        op=OP.add)
                    ex = spool.tile([P, SLOTS_PER_CHUNK * H], BF, tag="ex", name="ex")
                    nc.scalar.activation(
                        out=ex[:, 0:S * H], in_=s_t[:, 0:S * H],
                        func=mybir.ActivationFunctionType.Exp)
                    denom = spool.tile([P, CT_MAX * H], FP, tag="dn", name="dn")
                    nc.vector.reduce_sum(
                        denom[:, 0:ct * H].rearrange("p (t h) -> p t h", h=H),
                        ex[:, 0:S * H].rearrange(
                            "p (t w h) -> p t h w", w=w, h=H),
                        axis=AX.X)
                    rec = spool.tile([P, CT_MAX * H], FP, tag="rc", name="rc")
                    nc.vector.reciprocal(rec[:, 0:ct * H], denom[:, 0:ct * H])
                    # weighted v
                    nc.vector.tensor_tensor(
                        out=tmp[:, 0:S * D].rearrange(
                            "p (s h e) -> p s h e", h=H, e=HD),
                        in0=qv_g[:, 0:S * 2 * D].rearrange(
                            "p (s c) -> p s c", c=2 * D)[:, :, D:2 * D]
                            .rearrange("p s (h e) -> p s h e", h=H),
                        in1=ex[:, 0:S * H].rearrange("p (s h) -> p s h", h=H)
                            .unsqueeze(3).to_broadcast([P, S, H, HD]),
                        op=OP.mult)
                    # reduce over w via pairwise tree adds (contiguous runs
                    # beat the strided reduce_sum by ~4x on DVE)
                    agg = spool.tile([P, CT_MAX * D], FP, tag="agg", name="agg")
                    tview = tmp[:, 0:S * D].rearrange(
                        "p (t w d) -> p t w d", w=w, d=D)
                    n = w
                    while n > 2:
                        h = n // 2
                        nc.vector.tensor_tensor(
                            out=tview[:, :, 0:h, :], in0=tview[:, :, 0:h, :],
                            in1=tview[:, :, n - h:n, :], op=OP.add)
                        n -= h
                    if n == 2:
                        nc.vector.tensor_tensor(
                            out=agg[:, 0:ct * D].rearrange(
                                "p (t z d) -> p t z d", t=ct, z=1),
                            in0=tview[:, :, 0:1, :], in1=tview[:, :, 1:2, :],
                            op=OP.add)
                    else:
                        nc.vector.tensor_copy(
                            agg[:, 0:ct * D].rearrange(
                                "p (t z d) -> p t z d", t=ct, z=1),
                            tview[:, :, 0:1, :])
                    aggn = spool.tile([P, CT_MAX * D], BF, tag="aggn", name="aggn")
                    nc.vector.tensor_tensor(
                        out=aggn[:, 0:ct * D].rearrange(
                            "p (t h e) -> p t h e", h=H, e=HD),
                        in0=agg[:, 0:ct * D].rearrange(
                            "p (t h e) -> p t h e", h=H, e=HD),
                        in1=rec[:, 0:ct * H].rearrange("p (t h) -> p t h", h=H)
                            .unsqueeze(3).to_broadcast([P, ct, H, HD]),
                        op=OP.mult)
                    # fused output projection, one node tile at a time
                    for tl in range(ct):
                        nt = nt0 + tl
                        aT_ps = pspool_o.tile([P, P], BF, tag="aT", name="aT")
                        nc.tensor.transpose(
                            aT_ps[:], aggn[:, tl * D:(tl + 1) * D], ident[:])
                        aT = projpool.tile([P, P], BF, tag="aT_sb", name="aT_sb")
                        nc.vector.tensor_copy(aT[:], aT_ps[:])
                        if l < L - 1:
                            oT_ps = pspool_o.tile([P, P], FP, tag="oT", name="oT")
                            nc.tensor.matmul(
                                oT_ps[:], lhsT=wo_sb[:, l * D:(l + 1) * D],
                                rhs=aT[:], start=True, stop=True)
                            nc.vector.tensor_tensor(
                                out=xT_sb[:, nt * P:(nt + 1) * P],
                                in0=oT_ps[:],
                                in1=boT_sb[:, l:l + 1].to_broadcast([P, P]),
                                op=OP.add)
                        else:
                            o_ps = pspool_o.tile([P, nclass], FP, tag="o", name="o")
                            nc.tensor.matmul(
                                o_ps[:], lhsT=aT[:],
                                rhs=wo_sb[:, l * D:l * D + nclass],
                                start=True, stop=True)
                            o_sb = projpool.tile(
                                [P, nclass], FP, tag="o_sb", name="o_sb")
                            nc.vector.tensor_tensor(
                                out=o_sb[:], in0=o_ps[:], in1=bol_sb[:],
                                op=OP.add)
                            nc.sync.dma_start(
                                out_ext[nt * P:(nt + 1) * P, :], o_sb[:])
                    ecol += (128 * S) // 16
                    scol += S
                # tail tiles (deg-0 + padding rows): out = bias only
                for t in range(lay.n_class_tiles, T):
                    if l < L - 1:
                        nc.vector.tensor_copy(
                            xT_sb[:, t * P:(t + 1) * P],
                            boT_sb[:, l:l + 1].to_broadcast([P, P]))
                    else:
                        o_sb = projpool.tile(
                            [P, nclass], FP, tag="o_sb", name="o_sb")
                        nc.vector.tensor_copy(o_sb[:], bol_sb[:])
                        nc.sync.dma_start(out_ext[t * P:(t + 1) * P, :], o_sb[:])
    nc.compile()
    return nc


# ----------------------------------------------------------------------------
# Entry point
# ----------------------------------------------------------------------------

_trace = [False]  # test.py can flip this to profile


def kernel(x, src, dst, qkv_w, qkv_b, out_w, out_b, out_w_last, out_b_last):
    x = np.asarray(x, dtype=np.float32)
    lay = build_layout(np.asarray(src), np.asarray(dst), x.shape[0])
    nclass = np.asarray(out_w_last).shape[1]
    in_maps = host_inputs(
        lay, x, np.asarray(qkv_w, dtype=np.float32),
        np.asarray(qkv_b, dtype=np.float32), np.asarray(out_w, dtype=np.float32),
        np.asarray(out_b, dtype=np.float32),
        np.asarray(out_w_last, dtype=np.float32),
        np.asarray(out_b_last, dtype=np.float32),
    )
    nc = build_nc(lay, nclass)
    res = run_bass_kernel_spmd(
        nc, in_maps, core_ids=list(range(NCORES)), trace=_trace[0]
    )
    kernel.last_results = res
    outs = [np.asarray(res.results[c]["out"], dtype=np.float32)
            for c in range(NCORES)]
    return host_output(lay, outs, nclass)


# revision 20
# speedup vs baseline: 1.8923x; 1.1257x over previous
"""Graph-Transformer message-passing kernel for 8 Trainium2 NeuronCores.

Strategy (v2, dst-shard + batched SWDGE gathers, bf16):
  - Nodes split into 8 contiguous dst ranges; core c owns all edges into its
    range, so softmax/aggregation are fully local to a core.
  - Host groups each core's nodes into degree classes (DP-optimized widths),
    pads each node's in-edge list to the class width W; pad slots carry a
    -1e30 mask so they vanish in the softmax.
  - Per layer: each core projects Q|K|V for its rows (PE, bf16), the q|v
    halves are AllGathered into a global [8R, 256] bf16 table, each core then
    compacts the ~31.7K unique source rows it needs into a private <32768-row
    table (two dma_gather passes, int16 idx limit), and edge-slot q|v rows are
    batch-gathered from it (one dma_gather per ~32-slot chunk instead of one
    indirect DMA per edge slot: ~1us fixed SWDGE cost amortized 32x).
  - Masked softmax (no max-subtraction: |score| < 4 for this model family) and
    weighted aggregation run on DVE in bf16; output projection on PE keeps x
    feature-major (xT) so the next layer's QKV needs no transpose.
  - 3 layers fused in one NEFF; host inverts the node permutation.
"""

import numpy as np
import ml_dtypes

import concourse.bass as bass
import concourse.bacc as bacc
import concourse.mybir as mybir
import concourse.tile as tile
from concourse.masks import make_identity
from concourse.bass_utils import run_bass_kernel_spmd

NCORES = 8
L = 3
H = 8
D = 128
HD = D // H
SCALE = 1.0 / float(np.sqrt(HD))
NEG = -1.0e30
P = 128
LO_ROWS = 32768          # int16 idx limit for the low gather region
SLOTS_PER_CHUNK = 32     # edge-slot columns per DVE chunk
COMP_CHUNK = 1024        # rows per compaction dma_gather (SWDGE ring: <=65 descs/ring)
GATHER_MAX_SLOTS = 8     # 128*8 = 1024 idxs per edge dma_gather

FP = mybir.dt.float32
BF = mybir.dt.bfloat16
I16 = mybir.dt.int16
AX = mybir.AxisListType
OP = mybir.AluOpType
BFNP = ml_dtypes.bfloat16


def _wrap16(idxs):
    """Lay out idx stream positions j -> [j%16, j//16], replicated across all
    eight 16-partition groups (the Q7 ucode reads group 16..31 on queue 0)."""
    n = len(idxs)
    cols = (n + 15) // 16
    t = np.zeros((16, cols), dtype=np.int16)
    t[np.arange(n) % 16, np.arange(n) // 16] = idxs
    return np.tile(t, (8, 1))


class Layout:
    pass


def _pick_classes(deg, chunk, n_nodes):
    """DP over degree boundaries minimizing total padded slots."""
    dmax = int(deg.max())
    counts = np.zeros((NCORES, dmax + 1), dtype=np.int64)
    for c in range(NCORES):
        d = deg[c * chunk:min(n_nodes, (c + 1) * chunk)]
        counts[c] = np.bincount(d, minlength=dmax + 1)
    cum = counts.cumsum(axis=1)  # cum[c, w] = nodes with deg <= w

    def cost(lo, w):  # class covers degrees (lo, w]
        n = cum[:, w] - cum[:, lo]
        cap = ((n + P - 1) // P * P).max()
        return int(cap) * w

    INF = float("inf")
    best = [0.0] + [INF] * dmax
    prev = [0] * (dmax + 1)
    for w in range(1, dmax + 1):
        for lo in range(w):
            v = best[lo] + cost(lo, w)
            if v < best[w]:
                best[w], prev[w] = v, lo
    bounds = []
    w = dmax
    while w > 0:
        bounds.append(w)
        w = prev[w]
    return sorted(bounds)


def build_layout(src, dst, n_nodes):
    src = np.asarray(src).astype(np.int64)
    dst = np.asarray(dst).astype(np.int64)
    N = n_nodes
    chunk = (N + NCORES - 1) // NCORES

    deg = np.bincount(dst, minlength=N)
    order = np.argsort(dst, kind="stable")
    src_sorted = src[order]
    starts = np.zeros(N + 1, dtype=np.int64)
    np.cumsum(deg, out=starts[1:])

    classes = _pick_classes(deg, chunk, N)

    # per-core per-class node lists + uniform caps
    node_lists = {}
    for c in range(NCORES):
        nlo, nhi = c * chunk, min(N, (c + 1) * chunk)
        d = deg[nlo:nhi]
        lo = 0
        for w in classes:
            node_lists[(c, w)] = np.nonzero((d > lo) & (d <= w))[0] + nlo
            lo = w
    caps = {}
    for w in classes:
        cap = max(len(node_lists[(c, w)]) for c in range(NCORES))
        caps[w] = ((cap + P - 1) // P) * P

    n_class_rows = sum(caps.values())
    deg0_max = max(
        int((deg[c * chunk:min(N, (c + 1) * chunk)] == 0).sum())
        for c in range(NCORES)
    )
    R = ((n_class_rows + deg0_max + P - 1) // P) * P
    T = R // P

    base = {}
    b = 0
    for w in classes:
        base[w] = b
        b += caps[w]

    perm = np.full((NCORES, R), -1, dtype=np.int64)
    row_of = np.full(N, -1, dtype=np.int64)
    for c in range(NCORES):
        nlo, nhi = c * chunk, min(N, (c + 1) * chunk)
        for w in classes:
            nl = node_lists[(c, w)]
            perm[c, base[w]:base[w] + len(nl)] = nl
            row_of[nl] = c * R + base[w] + np.arange(len(nl))
        deg0 = np.nonzero(deg[nlo:nhi] == 0)[0] + nlo
        perm[c, n_class_rows:n_class_rows + len(deg0)] = deg0
        row_of[deg0] = c * R + n_class_rows + np.arange(len(deg0))

    # split rows into NREG regions so (a) each AllGather part overlaps other
    # work and (b) each part-table stays below the int16 idx limit
    NREG = 4
    tile_cnt = [T // NREG + (1 if i < T % NREG else 0) for i in range(NREG)]
    treg0 = np.cumsum([0] + tile_cnt)  # tile offsets per region
    hrows = [tc_ * P for tc_ in tile_cnt]
    hbase = [int(treg0[i]) * P for i in range(NREG)]
    assert all(NCORES * h <= 32767 for h in hrows)

    # edge-slot chunks (uniform across cores): (w, nt0, t_in_class, ct)
    chunks = []
    nt = 0
    for w in classes:
        tiles = caps[w] // P
        ct0 = max(1, SLOTS_PER_CHUNK // w)
        t = 0
        while t < tiles:
            ct = min(ct0, tiles - t)
            chunks.append((w, nt, t, ct))
            nt += ct
            t += ct
    n_class_tiles = nt

    # per-core: unique src rows -> compact table; edge idx + mask tables.
    # Unique rows are split by which half-table (A/B) holds them; idx values
    # are half-local (core*half_rows + local).
    uniq = [[] for _ in range(NREG)]  # per region: list over cores of (rows, halfidx)
    for c in range(NCORES):
        sel = (dst >= c * chunk) & (dst < (c + 1) * chunk)
        rows = np.unique(row_of[src[sel]])
        local = rows % R
        core = rows // R
        for i in range(NREG):
            inr = (local >= hbase[i]) & (local < hbase[i] + hrows[i])
            uniq[i].append(
                (rows[inr], core[inr] * hrows[i] + local[inr] - hbase[i]))
    NR = [max((len(u[0]) + P - 1) // P * P for u in uniq[i])
          for i in range(NREG)]
    NU = sum(NR)
    assert NU <= 32767, f"compact table {NU} exceeds int16"
    nrbase = np.cumsum([0] + NR)

    comp_idx = np.zeros((NCORES, NU), dtype=np.int64)
    cpos = np.zeros((NCORES, NCORES * R), dtype=np.int32)
    for c in range(NCORES):
        for i in range(NREG):
            rows_i, half_i = uniq[i][c]
            comp_idx[c, nrbase[i]:nrbase[i] + len(half_i)] = half_i
            cpos[c, rows_i] = nrbase[i] + np.arange(len(rows_i))

    # comp gather chunk list: (region, n, tc_base, col_ofs)
    comp_chunks = []
    for i in range(NREG):
        done = 0
        while done < NR[i]:
            n = min(COMP_CHUNK, NR[i] - done)
            b = int(nrbase[i]) + done
            comp_chunks.append((i, n, b, b // 16))
            done += n

    # edge idx + mask, chunk-slot-major: j = s*128 + p, s = t_loc*w + ws
    S_tot = sum(w * ct for (w, _, _, ct) in chunks)
    eidx = np.zeros((NCORES, 128 * S_tot), dtype=np.int64)
    emask = np.full((NCORES, P, S_tot), NEG, dtype=np.float32)
    for c in range(NCORES):
        # per-class [cap, w] idx/valid matrices, vectorized over nodes
        mats = {}
        for w in classes:
            cap = caps[w]
            nodes = perm[c, base[w]:base[w] + cap]
            nd = np.maximum(nodes, 0)
            dg = np.where(nodes >= 0, deg[nd], 0)
            im = np.zeros((cap, w), dtype=np.int64)
            vm = np.zeros((cap, w), dtype=bool)
            for ws in range(w):
                ok = ws < dg
                sidx = starts[nd] + ws
                im[ok, ws] = cpos[c, row_of[src_sorted[np.minimum(
                    sidx, len(src_sorted) - 1)]]][ok]
                vm[:, ws] = ok
            vm[nodes < 0, 0] = True  # pad row: 1 live slot, no NaN
            mats[w] = (im, vm)
        j0 = 0
        s0 = 0
        for (w, nt0, t_in_class, ct) in chunks:
            im, vm = mats[w]
            blk = slice(t_in_class * P, (t_in_class + ct) * P)
            imc = im[blk].reshape(ct, P, w)
            vmc = vm[blk].reshape(ct, P, w)
            n = ct * w * P
            eidx[c, j0:j0 + n] = imc.transpose(0, 2, 1).ravel()
            emask[c][:, s0:s0 + ct * w] = np.where(
                vmc.transpose(1, 0, 2).reshape(P, ct * w), 0.0, NEG)
            j0 += 128 * ct * w
            s0 += ct * w
    lay = Layout()
    lay.N, lay.R, lay.T, lay.chunk = N, R, T, chunk
    lay.classes, lay.caps, lay.base = classes, caps, base
    lay.n_class_tiles = n_class_tiles
    lay.chunks, lay.comp_chunks = chunks, comp_chunks
    lay.NU, lay.S_tot = NU, S_tot
    lay.NREG, lay.tile_cnt, lay.treg0, lay.hrows = NREG, tile_cnt, treg0, hrows
    lay.perm, lay.row_of = perm, row_of
    lay.eidx, lay.emask, lay.comp_idx = eidx, emask, comp_idx
    return lay


def host_inputs(lay, x, qkv_w, qkv_b, out_w, out_b, out_w_last, out_b_last):
    x = np.asarray(x, dtype=np.float32)
    nclass = out_w_last.shape[1]
    wc = np.zeros((L, D, 3 * D), dtype=np.float32)
    bc = np.zeros((L, P, 3 * D), dtype=np.float32)
    wo = np.zeros((L, D, D), dtype=np.float32)
    boT = np.zeros((D, L), dtype=np.float32)
    bo_last = np.tile(out_b_last[None, :], (P, 1)).astype(np.float32)
    for l in range(L):
        wq, wk, wv = qkv_w[l, 0], qkv_w[l, 1], qkv_w[l, 2]
        bq, bk, bv = qkv_b[l, 0], qkv_b[l, 1], qkv_b[l, 2]
        wc[l] = np.concatenate([wq, wk * SCALE, wv], axis=1)
        bc[l] = np.tile(np.concatenate([bq, bk * SCALE, bv])[None, :], (P, 1))
        if l < L - 1:
            wo[l] = out_w[l]
            boT[:, l] = out_b[l]
        else:
            wo[l, :, :nclass] = out_w_last

    in_maps = []
    for c in range(NCORES):
        xp = np.where((lay.perm[c] >= 0)[:, None],
                      x[np.maximum(lay.perm[c], 0)], 0.0)
        m = {
            "x0T": np.ascontiguousarray(xp.T).astype(BFNP),
            "wc": wc.astype(BFNP), "bc": bc,
            "wo": wo.astype(BFNP), "boT": boT, "bo_last": bo_last,
            "eidx": _wrap16(lay.eidx[c]),
            "emask": lay.emask[c],
            "cidx": _wrap16(lay.comp_idx[c]),
        }
        in_maps.append(m)
    return in_maps


def host_output(lay, outs, nclass):
    full = np.zeros((lay.N, nclass), dtype=np.float32)
    for c in range(NCORES):
        real = lay.perm[c] >= 0
        full[lay.perm[c][real]] = outs[c][real]
    return full


# ----------------------------------------------------------------------------
# Device program
# ----------------------------------------------------------------------------

_PHASES = ["ABCD"]  # debug: phase bisect knob


def build_nc(lay, nclass):
    R, T = lay.R, lay.T
    nc = bacc.Bacc(trn_type="TRN2", num_devices=NCORES, num_swdge_queues=4)

    x0T = nc.dram_tensor("x0T", [D, R], BF, kind="ExternalInput")
    wc = nc.dram_tensor("wc", [L, D, 3 * D], BF, kind="ExternalInput")
    bc = nc.dram_tensor("bc", [L, P, 3 * D], FP, kind="ExternalInput")
    wo = nc.dram_tensor("wo", [L, D, D], BF, kind="ExternalInput")
    boT = nc.dram_tensor("boT", [D, L], FP, kind="ExternalInput")
    bo_last = nc.dram_tensor("bo_last", [P, nclass], FP, kind="ExternalInput")
    eidx_d = nc.dram_tensor(
        "eidx", [128, (128 * lay.S_tot) // 16], I16, kind="ExternalInput")
    emask_d = nc.dram_tensor("emask", [P, lay.S_tot], FP, kind="ExternalInput")
    cidx_d = nc.dram_tensor(
        "cidx", [128, lay.NU // 16], I16, kind="ExternalInput")
    out_ext = nc.dram_tensor("out", [R, nclass], FP, kind="ExternalOutput")

    NREG, tile_cnt, treg0, hrows = lay.NREG, lay.tile_cnt, lay.treg0, lay.hrows
    qv_slices = [
        nc.dram_tensor(f"qv_slice{i}", [hrows[i], 2 * D], BF, kind="Internal")
        for i in range(NREG)]
    qv_fulls = [
        nc.dram_tensor(f"qv_full{i}", [NCORES * hrows[i], 2 * D], BF,
                       kind="Internal", addr_space="Shared")
        for i in range(NREG)]
    tcomp = nc.dram_tensor("tcomp", [lay.NU, 2 * D], BF, kind="Internal")
    rg = [list(range(NCORES))]

    CT_MAX = max(ct for (_, _, _, ct) in lay.chunks)
    with tile.TileContext(nc) as tc:
        with (
            tc.tile_pool(name="const", bufs=1) as cpool,
            tc.tile_pool(name="persist", bufs=1) as ppool,
            tc.tile_pool(name="proj", bufs=4) as projpool,
            tc.tile_pool(name="work", bufs=3) as wpool,
            tc.tile_pool(name="small", bufs=4) as spool,
            tc.tile_pool(name="psum", bufs=2, space="PSUM") as pspool,
            tc.tile_pool(name="psum_o", bufs=2, space="PSUM") as pspool_o,
        ):
            ident = cpool.tile([P, P], BF, tag="ident", name="ident")
            make_identity(nc, ident[:])
            wc_sb = cpool.tile([P, L * 3 * D], BF, tag="wc", name="wc")
            nc.sync.dma_start(
                wc_sb[:].rearrange("k (l n) -> k l n", l=L),
                wc[:].rearrange("l k n -> k l n"))
            bc_sb = cpool.tile([P, L * 3 * D], FP, tag="bc", name="bc")
            nc.sync.dma_start(
                bc_sb[:].rearrange("p (l n) -> p l n", l=L),
                bc[:].rearrange("l p n -> p l n"))
            wo_sb = cpool.tile([P, L * D], BF, tag="wo", name="wo")
            nc.sync.dma_start(
                wo_sb[:].rearrange("k (l n) -> k l n", l=L),
                wo[:].rearrange("l k n -> k l n"))
            boT_sb = cpool.tile([P, L], FP, tag="boT", name="boT")
            nc.sync.dma_start(boT_sb[:], boT[:])
            bol_sb = cpool.tile([P, nclass], FP, tag="bol", name="bol")
            nc.sync.dma_start(bol_sb[:], bo_last[:])
            eidx_sb = cpool.tile(
                [128, (128 * lay.S_tot) // 16], I16, tag="eidx", name="eidx")
            nc.sync.dma_start(eidx_sb[:], eidx_d[:])
            mask_sb = cpool.tile([P, lay.S_tot], FP, tag="mask", name="mask")
            nc.sync.dma_start(mask_sb[:], emask_d[:])
            cidx_sb = cpool.tile([128, lay.NU // 16], I16, tag="cidx", name="cidx")
            nc.sync.dma_start(cidx_sb[:], cidx_d[:])

            qrr = [0]  # SWDGE queue round-robin

            def next_q():
                qrr[0] = (qrr[0] + 1) % 4
                return qrr[0]

            xT_sb = ppool.tile([P, R], BF, tag="x", name="x")
            nc.sync.dma_start(xT_sb[:], x0T[:])
            k_sb = ppool.tile([P, T * D], BF, tag="k", name="k")
            tc.strict_bb_all_engine_barrier()

            for l in range(L):
                # ---- Phase A: QKV projection (x kept feature-major) ----
                def proj_tile(t):
                    qkv_ps = pspool.tile([P, 3 * D], FP, tag="qkv", name="qkv")
                    nc.tensor.matmul(
                        qkv_ps[:], lhsT=xT_sb[:, t * P:(t + 1) * P],
                        rhs=wc_sb[:, l * 3 * D:(l + 1) * 3 * D],
                        start=True, stop=True,
                    )
                    bofs = l * 3 * D
                    qv_st = projpool.tile([P, 2 * D], BF, tag="qv_st", name="qv_st")
                    nc.vector.tensor_tensor(
                        out=qv_st[:, 0:D], in0=qkv_ps[:, 0:D],
                        in1=bc_sb[:, bofs:bofs + D], op=OP.add)
                    nc.vector.tensor_tensor(
                        out=qv_st[:, D:2 * D], in0=qkv_ps[:, 2 * D:3 * D],
                        in1=bc_sb[:, bofs + 2 * D:bofs + 3 * D], op=OP.add)
                    nc.vector.tensor_tensor(
                        out=k_sb[:, t * D:(t + 1) * D], in0=qkv_ps[:, D:2 * D],
                        in1=bc_sb[:, bofs + D:bofs + 2 * D], op=OP.add)
                    ri = int(np.searchsorted(treg0, t, side="right")) - 1
                    tl0 = t - int(treg0[ri])
                    nc.sync.dma_start(
                        qv_slices[ri][tl0 * P:(tl0 + 1) * P, :], qv_st[:])

                def comp_region(region):
                    # compact the unique src rows of one part-table
                    for (rgn, n, tc_base, col_ofs) in (
                            lay.comp_chunks if "C" in _PHASES[0] else []):
                        if rgn != region:
                            continue
                        cg = wpool.tile(
                            [P, COMP_CHUNK // P * 2 * D], BF, tag="cg", name="cg")
                        sl = n // P * 2 * D
                        nc.gpsimd.dma_gather(
                            cg[:, 0:sl].rearrange("p (s d) -> p s d", d=2 * D),
                            qv_fulls[region][:, :],
                            cidx_sb[:, col_ofs:col_ofs + n // 16],
                            n, n, 2 * D, queue_num=next_q(),
                        )
                        nc.scalar.dma_start(
                            tcomp[tc_base:tc_base + n, :].rearrange(
                                "(s p) d -> p s d", p=P),
                            cg[:, 0:sl].rearrange("p (s d) -> p s d", d=2 * D),
                        )

                # projection regions interleaved with AllGathers + compaction:
                # CC_i is issued right after region i's tiles project; its
                # compaction gathers then overlap later regions' CC transfers
                for i in range(NREG):
                    for t in range(int(treg0[i]), int(treg0[i + 1])):
                        proj_tile(t)
                    nc.gpsimd.collective_compute(
                        "AllGather", OP.bypass, replica_groups=rg,
                        ins=[qv_slices[i][:, :]], outs=[qv_fulls[i][:, :]],
                    )
                    comp_region(i)

                # ---- Phase D: edge softmax + aggregation + out projection ----
                ecol = 0
                scol = 0
                for (w, nt0, t_in_class, ct) in (
                        lay.chunks if "D" in _PHASES[0] else []):
                    S = ct * w
                    qv_g = wpool.tile(
                        [P, SLOTS_PER_CHUNK * 2 * D], BF, tag="qvg", name="qvg")
                    off = 0
                    while off < S:
                        gs = min(GATHER_MAX_SLOTS, S - off)
                        nc.gpsimd.dma_gather(
                            qv_g[:, off * 2 * D:(off + gs) * 2 * D]
                                .rearrange("p (s d) -> p s d", d=2 * D),
                            tcomp[:, :],
                            eidx_sb[:, ecol + off * 8:ecol + (off + gs) * 8],
                            128 * gs, 128 * gs, 2 * D, queue_num=next_q(),
                        )
                        off += gs
                    # scores: per-head dot(q_gathered, k_local)
                    tmp = wpool.tile(
                        [P, SLOTS_PER_CHUNK * D], BF, tag="tmp", name="tmp")
                    q4 = qv_g[:, 0:S * 2 * D].rearrange(
                        "p (t w c) -> p t w c", w=w, c=2 * D)[:, :, :, 0:D]
                    k4 = (k_sb[:, nt0 * D:(nt0 + ct) * D]
                          .rearrange("p (t d) -> p t d", d=D)
                          .unsqueeze(2).to_broadcast([P, ct, w, D]))
                    nc.vector.tensor_tensor(
                        out=tmp[:, 0:S * D].rearrange(
                            "p (t w d) -> p t w d", w=w, d=D),
                        in0=q4, in1=k4, op=OP.mult)
                    s_t = spool.tile([P, SLOTS_PER_CHUNK * H], FP, tag="s", name="s")
                    nc.vector.reduce_sum(
                        s_t[:, 0:S * H].rearrange("p (s h) -> p s h", h=H),
                        tmp[:, 0:S * D].rearrange(
                            "p (s h e) -> p s h e", h=H, e=HD),
                        axis=AX.X)
                    nc.vector.tensor_tensor(
                        out=s_t[:, 0:S * H].rearrange("p (s h) -> p s h", h=H),
                        in0=s_t[:, 0:S * H].rearrange("p (s h) -> p s h", h=H),
                        in1=mask_sb[:, scol:scol + S]
                            .unsqueeze(2).to_broadcast([P, S, H]),
                        op=OP.add)
                    ex = spool.tile([P, SLOTS_PER_CHUNK * H], BF, tag="ex", name="ex")
                    nc.scalar.activation(
                        out=ex[:, 0:S * H], in_=s_t[:, 0:S * H],
                        func=mybir.ActivationFunctionType.Exp)
                    denom = spool.tile([P, CT_MAX * H], FP, tag="dn", name="dn")
                    nc.vector.reduce_sum(
                        denom[:, 0:ct * H].rearrange("p (t h) -> p t h", h=H),
                        ex[:, 0:S * H].rearrange(
                            "p (t w h) -> p t h w", w=w, h=H),
                        axis=AX.X)
                    rec = spool.tile([P, CT_MAX * H], FP, tag="rc", name="rc")
                    nc.vector.reciprocal(rec[:, 0:ct * H], denom[:, 0:ct * H])
                    # weighted v
                    nc.vector.tensor_tensor(
                        out=tmp[:, 0:S * D].rearrange(
                            "p (s h e) -> p s h e", h=H, e=HD),
                        in0=qv_g[:, 0:S * 2 * D].rearrange(
                            "p (s c) -> p s c", c=2 * D)[:, :, D:2 * D]
                            .rearrange("p s (h e) -> p s h e", h=H),
                        in1=ex[:, 0:S * H].rearrange("p (s h) -> p s h", h=H)
                            .unsqueeze(3).to_broadcast([P, S, H, HD]),
                        op=OP.mult)
                    # reduce over w via pairwise tree adds (contiguous runs
                    # beat the strided reduce_sum by ~4x on DVE)
                    agg = spool.tile([P, CT_MAX * D], FP, tag="agg", name="agg")
                    tview = tmp[:, 0:S * D].rearrange(
                        "p (t w d) -> p t w d", w=w, d=D)
                    n = w
                    while n > 2:
                        h = n // 2
                        nc.vector.tensor_tensor(
                            out=tview[:, :, 0:h, :], in0=tview[:, :, 0:h, :],
                            in1=tview[:, :, n - h:n, :], op=OP.add)
                        n -= h
                    if n == 2:
                        nc.vector.tensor_tensor(
                            out=agg[:, 0:ct * D].rearrange(
                                "p (t z d) -> p t z d", t=ct, z=1),
                            in0=tview[:, :, 0:1, :], in1=tview[:, :, 1:2, :],
                            op=OP.add)
                    else:
                        nc.vector.tensor_copy(
                            agg[:, 0:ct * D].rearrange(
                                "p (t z d) -> p t z d", t=ct, z=1),
                            tview[:, :, 0:1, :])
                    aggn = spool.tile([P, CT_MAX * D], BF, tag="aggn", name="aggn")
                    nc.vector.tensor_tensor(
                        out=aggn[:, 0:ct * D].rearrange(
                            "p (t h e) -> p t h e", h=H, e=HD),
                        in0=agg[:, 0:ct * D].rearrange(
                            "p (t h e) -> p t h e", h=H, e=HD),
                        in1=rec[:, 0:ct * H].rearrange("p (t h) -> p t h", h=H)
                            .unsqueeze(3).to_broadcast([P, ct, H, HD]),
                        op=OP.mult)
                    # fused output projection, one node tile at a time
                    for tl in range(ct):
                        nt = nt0 + tl
                        aT_ps = pspool_o.tile([P, P], BF, tag="aT", name="aT")
                        nc.tensor.transpose(
                            aT_ps[:], aggn[:, tl * D:(tl + 1) * D], ident[:])
                        aT = projpool.tile([P, P], BF, tag="aT_sb", name="aT_sb")
                        nc.vector.tensor_copy(aT[:], aT_ps[:])
                        if l < L - 1:
                            oT_ps = pspool_o.tile([P, P], FP, tag="oT", name="oT")
                            nc.tensor.matmul(
                                oT_ps[:], lhsT=wo_sb[:, l * D:(l + 1) * D],
                                rhs=aT[:], start=True, stop=True)
                            nc.vector.tensor_tensor(
                                out=xT_sb[:, nt * P:(nt + 1) * P],
                                in0=oT_ps[:],
                                in1=boT_sb[:, l:l + 1].to_broadcast([P, P]),
                                op=OP.add)
                        else:
                            o_ps = pspool_o.tile([P, nclass], FP, tag="o", name="o")
                            nc.tensor.matmul(
                                o_ps[:], lhsT=aT[:],
                                rhs=wo_sb[:, l * D:l * D + nclass],
                                start=True, stop=True)
                            o_sb = projpool.tile(
                                [P, nclass], FP, tag="o_sb", name="o_sb")
                            nc.vector.tensor_tensor(
                                out=o_sb[:], in0=o_ps[:], in1=bol_sb[:],
                                op=OP.add)
                            nc.sync.dma_start(
                                out_ext[nt * P:(nt + 1) * P, :], o_sb[:])
                    ecol += (128 * S) // 16
                    scol += S
                # tail tiles (deg-0 + padding rows): out = bias only
                for t in range(lay.n_class_tiles, T):
                    if l < L - 1:
                        nc.vector.tensor_copy(
                            xT_sb[:, t * P:(t + 1) * P],
                            boT_sb[:, l:l + 1].to_broadcast([P, P]))
                    else:
                        o_sb = projpool.tile(
                            [P, nclass], FP, tag="o_sb", name="o_sb")
                        nc.vector.tensor_copy(o_sb[:], bol_sb[:])
                        nc.sync.dma_start(out_ext[t * P:(t + 1) * P, :], o_sb[:])
    nc.compile()
    return nc


# ----------------------------------------------------------------------------
# Entry point
# ----------------------------------------------------------------------------

_trace = [False]  # test.py can flip this to profile


def kernel(x, src, dst, qkv_w, qkv_b, out_w, out_b, out_w_last, out_b_last):
    x = np.asarray(x, dtype=np.float32)
    lay = build_layout(np.asarray(src), np.asarray(dst), x.shape[0])
    nclass = np.asarray(out_w_last).shape[1]
    in_maps = host_inputs(
        lay, x, np.asarray(qkv_w, dtype=np.float32),
        np.asarray(qkv_b, dtype=np.float32), np.asarray(out_w, dtype=np.float32),
        np.asarray(out_b, dtype=np.float32),
        np.asarray(out_w_last, dtype=np.float32),
        np.asarray(out_b_last, dtype=np.float32),
    )
    nc = build_nc(lay, nclass)
    res = run_bass_kernel_spmd(
        nc, in_maps, core_ids=list(range(NCORES)), trace=_trace[0]
    )
    kernel.last_results = res
    outs = [np.asarray(res.results[c]["out"], dtype=np.float32)
            for c in range(NCORES)]
    return host_output(lay, outs, nclass)
